# revision 37
# baseline (speedup 1.0000x reference)
"""Trainium2 Bass kernel for Baichuan attention (B=2, S=2048, H=4096, 32 heads).

Sharding: 8 cores = 2 (batch) x 4 (head groups of 8 heads), tensor-parallel
mirror of ColumnParallelLinear/RowParallelLinear. Each core computes, for its
batch b and head group g:
    qkT   = (w_pack q,k slice) @ x_b.T        [2048 qkdims, 2048 seq]
            in fp8e4 DoubleRow (x,w scaled by 512 each; descale folded into
            the cos/sin rope tables), rope on the DVE via a signed
            half-swap read (partition-offset APs + sign-folded sin table)
    v     = x_b @ (w_pack v slice).T          [2048 seq, 1024]  (bf16),
            written straight into a persistent SBUF tile (no DRAM roundtrip)
    per head: scoresT tiles -> exp -> causal mask
              l = ones @ eP (softmax denominators; full tiles pre-summed in
              quads on the DVE so the PE does 1/4 of the ones-matmuls).
              All l-matmuls of a group are DEFERRED into the next group's
              emission so the PE never stalls on the DVE adds/masks at a
              group boundary.
              out_hT = sum v eP
    partial = attn_out @ w_o[:, cols].T       [2048, 4096]  (f32)
Host sums the 4 TP partials per batch (row-parallel all-reduce done on host).

DMA layout: two HWDGE queues only (Sync + Scalar engines). Weight panels ride
Sync; bulk x slabs, cos/sin tables and the o-proj weight panels ride Scalar so
neither stream head-of-line-blocks the other. SBUF input tiles are split into
quarter/half tiles so the first consumer matmul waits only on the first
~256-512KB of DMA, not a whole 2-4MB slab (dependency tracking is
tile-granular).

Self-contained: hardcodes all shapes; only needs concourse + numpy + ml_dtypes.
"""
import math
from contextlib import ExitStack

import numpy as np
import ml_dtypes

import concourse.bass as bass
import concourse.mybir as mybir
import concourse.tile as tile
from concourse import bacc
from concourse.bass_utils import run_bass_kernel_spmd

bf16 = ml_dtypes.bfloat16
f8 = ml_dtypes.float8_e4m3
FP32 = mybir.dt.float32
BF16 = mybir.dt.bfloat16
F8E4 = mybir.dt.float8e4
DR = mybir.MatmulPerfMode.DoubleRow

B, S, H = 2, 2048, 4096
NH_TOT, HD = 32, 128
NHL = 8                # heads per core
KT = H // 128          # 32 contraction tiles for the projections
VD = NHL * HD          # 1024 local v dims
SCALE = 1.0 / math.sqrt(HD)
ROPE_BASE = 10000.0
SX = 512.0             # fp8 input scale for x
SW = 512.0             # fp8 input scale for w_pack qk rows
SQK = 32.0             # fp8 storage scale for rotated q,k
SCALE_Q = SCALE / (SQK * SQK)  # exp scale with the x32 qkrot descale folded in

_NC_CACHE = {}


def build_nc():
    nc = bacc.Bacc()
    x = nc.declare_dram_parameter("x", [2, 2, 128, KT, 512], BF16, isOutput=False)
    x8 = nc.declare_dram_parameter("x8", [2, 2, 128, KT, 512], F8E4, isOutput=False)
    wqk = nc.declare_dram_parameter("wqk", [16, 128, KT, 128], F8E4, isOutput=False)
    wv = nc.declare_dram_parameter("wv", [4, 128, 16, 512], BF16, isOutput=False)
    wo = nc.declare_dram_parameter("wo", [8, 128, 8, 512], BF16, isOutput=False)
    cos = nc.declare_dram_parameter("cos", [128, S], BF16, isOutput=False)
    sinn = nc.declare_dram_parameter("sinn", [128, S], BF16, isOutput=False)
    msk = nc.declare_dram_parameter("msk", [128, 128], BF16, isOutput=False)
    ones = nc.declare_dram_parameter("ones", [128, 128], BF16, isOutput=False)
    out = nc.declare_dram_parameter("out", [S, H], FP32, isOutput=True)

    EXP = mybir.ActivationFunctionType.Exp

    with tile.TileContext(nc) as tc, ExitStack() as g:
        glob = g.enter_context(tc.tile_pool(name="glob", bufs=1))

        # qkrot in fp8 (x32 scale, descale folded into the exp scale): the
        # qk-path quantization washes out through the near-uniform softmax,
        # and fp8 halves the dominant SBUF tensor (32KB vs 64KB)
        qkrot = [glob.tile([128, S], F8E4, tag=f"qkrot{t}", name=f"qkrot{t}")
                 for t in range(16)]
        v_all = glob.tile([128, 16, VD], BF16, tag="vall", name="vall")
        ones_sb = glob.tile([128, 128], BF16, tag="ones", name="ones_sb")

        # ---------- phase 1a: qk projection (fp8 DoubleRow) + rope ---------
        with ExitStack() as s1v:
            # v-phase pools live one scope up so their first loads can be
            # emitted mid-qk and prefetch during the qk tail
            xpoolv = s1v.enter_context(tc.tile_pool(name="xpv", bufs=1))
            wvpool = s1v.enter_context(tc.tile_pool(name="wvp", bufs=2))
            vxh = {}
            wvh = {}
            pool_holder = {}

            def load_vx(hf, xq, pool, tagid, engine=None):
                # 4 quarter tiles of [128, 8, 512] so the first v matmul
                # waits on 1MB, not the whole 4MB slab.  Distinct tag sets
                # (tagid) keep independent slabs off each other's WAR chains.
                quarters = []
                for qt in range(4):
                    qtile = pool.tile([128, 8, 512], BF16, tag=f"vxh{tagid}q{qt}",
                                      name=f"vxh{hf}{xq}q{qt}")
                    (engine or nc.scalar).dma_start(
                        out=qtile[:, :, :],
                        in_=x[hf, xq, :, qt * 8:(qt + 1) * 8, :],
                    )
                    quarters.append(qtile)
                vxh[(hf, xq)] = quarters
                return quarters

            def load_wv(hf, nb, kh):
                # wv panel for k-tiles [16*(2nb+kh), +16): two half tiles of
                # [128, 8, 512] (0.5MB DMA each) on the Sync queue
                halves = []
                for hh in range(2):
                    t = wvpool.tile([128, 8, 512], BF16, tag=f"wvp{hh}",
                                    name=f"wvp{hf}{nb}{kh}{hh}")
                    nc.sync.dma_start(out=t[:, :, :],
                                      in_=wv[2 * nb + kh, :, hh * 8:(hh + 1) * 8, :])
                    halves.append(t)
                wvh[(hf, nb, kh)] = halves
                return halves

            with ExitStack() as s1:
                c1 = s1.enter_context(tc.tile_pool(name="c1", bufs=1))
                xpool = s1.enter_context(tc.tile_pool(name="xp", bufs=1))
                wpool = s1.enter_context(tc.tile_pool(name="wp", bufs=6))
                evict = s1.enter_context(tc.tile_pool(name="ev", bufs=2))
                # bufs=1: the rope chain is DVE-only and the DVE is in-order,
                # so extra buffers cannot add overlap
                rope = s1.enter_context(tc.tile_pool(name="rope", bufs=1))
                pp = s1.enter_context(tc.tile_pool(name="pp", bufs=3, space="PSUM"))

                # one tile per 512-col chunk: rope flush sb only waits its own
                # chunk's DMA (dep tracking is tile-granular)
                cosc = [c1.tile([128, 512], BF16, tag=f"cosc{i}", name=f"cosc{i}")
                        for i in range(4)]
                sinnc = [c1.tile([128, 512], BF16, tag=f"sinnc{i}", name=f"sinnc{i}")
                         for i in range(4)]

                def load_rope_chunk(i, eng):
                    eng.dma_start(out=cosc[i][:], in_=cos[:, i * 512:(i + 1) * 512])
                    eng.dma_start(out=sinnc[i][:], in_=sinn[:, i * 512:(i + 1) * 512])

                wq_cache = {}

                def get_wqp(hf_, nb2_, mt_, hi_engine=None):
                    # two half-panels so the first matmul waits on 256KB only
                    key = (hf_, nb2_, mt_)
                    if key not in wq_cache:
                        lo = wpool.tile([128, 16, 128], F8E4, tag="wqplo",
                                        name=f"wqplo{hf_}{nb2_}{mt_}")
                        hi = wpool.tile([128, 16, 128], F8E4, tag="wqphi",
                                        name=f"wqphi{hf_}{nb2_}{mt_}")
                        nc.sync.dma_start(out=lo[:, :, :],
                                          in_=wqk[mt_, :, 0:16, :])
                        (hi_engine or nc.sync).dma_start(out=hi[:, :, :],
                                                         in_=wqk[mt_, :, 16:32, :])
                        wq_cache[key] = (lo, hi)
                    return wq_cache[key]

                x8_cache = {}

                def get_x8(hf_, xq_, engines=None):
                    # 4 quarter tiles [128, 8, 512] per 512-col slab; x loads
                    # ride the Scalar HWDGE queue in 4-ktile chunks so they
                    # never head-of-line-block the wqp panel stream (Sync).
                    # `engines` overrides the queue per quarter.
                    key = (hf_, xq_)
                    if key not in x8_cache:
                        tag = "xhlo" if xq_ == 0 else "xhhi"
                        quarters = []
                        for qt in range(4):
                            qtile = xpool.tile([128, 8, 512], F8E4,
                                               tag=f"{tag}q{qt}",
                                               name=f"x8{tag[2:]}{hf_}q{qt}")
                            eng = engines[qt] if engines else nc.scalar
                            for c in range(2):
                                eng.dma_start(
                                    out=qtile[:, c * 4:(c + 1) * 4, :],
                                    in_=x8[hf_, xq_, :, qt * 8 + c * 4:qt * 8 + (c + 1) * 4, :],
                                )
                            quarters.append(qtile)
                        x8_cache[key] = quarters
                    return x8_cache[key]

                # --- start-ramp preamble -------------------------------------
                # Neither queue alone can feed tiles 0-5 at PE pace (a panel
                # is 512KB/3.4us = one queue's line rate, and tile 0 needs the
                # whole 2MB lo slab), so split the ramp across BOTH queues.
                # (The Scalar engine's ACT_TABLE_LOAD delays its queue ~3us,
                # so the most-critical first bytes ride Sync.)
                get_wqp(0, 0, 0, hi_engine=nc.scalar)
                get_x8(0, 0, engines=[nc.sync, nc.scalar, nc.sync, nc.sync])
                for mt_pre in range(1, 6):
                    get_wqp(0, 0, mt_pre, hi_engine=nc.scalar)
                load_rope_chunk(0, nc.sync)
                nc.sync.dma_start(out=ones_sb[:], in_=ones[:])
                load_rope_chunk(1, nc.scalar)

                # partition p <- p+64 mod 128, in stream_shuffle's 4-partition
                # group units (32 groups, shift by 16)
                SWAP_MASK = [(gg + 16) % 32 for gg in range(32)]

                def flush_rope(qkraw, mt_p, sb_p):
                    # DVE-only rope: swap(q) is a signed row permutation of
                    # the SAME projection output — STREAM_SHUFFLE rotates the
                    # partitions by 64, and the sign rides the sinn table.
                    t1 = rope.tile([128, 512], BF16, tag="t1", name=f"t1_{mt_p}{sb_p}")
                    t2 = rope.tile([128, 512], BF16, tag="t2", name=f"t2_{mt_p}{sb_p}")
                    t2s = rope.tile([128, 512], BF16, tag="t2s", name=f"t2s_{mt_p}{sb_p}")
                    ci = sb_p // 512
                    nc.vector.stream_shuffle(t2s[:], qkraw[:], mask=SWAP_MASK)
                    nc.vector.tensor_mul(t2[:], t2s[:], sinnc[ci][:])
                    nc.vector.tensor_mul(t1[:], qkraw[:], cosc[ci][:])
                    nc.vector.tensor_add(qkrot[mt_p][:, sb_p:sb_p + 512], t1[:], t2[:])

                for hf in range(2):
                    xh_lo = get_x8(hf, 0)

                    # nb2-outer so each 512-col slab of xh has its last reader
                    # at the end of one sub-phase: the next half's x DMA for
                    # that slab overlaps the other slab's compute.  The hi
                    # slab's DMA is deferred to a mid-nb2=0 hook so the ramp
                    # traffic clears the Scalar queue first.
                    for nb2 in range(2):
                        for mt in range(16):
                            if nb2 == 0 and mt == 8:
                                if hf == 0:
                                    load_rope_chunk(2, nc.scalar)
                                    load_rope_chunk(3, nc.scalar)
                                get_x8(hf, 1)
                            wqlo, wqhi = get_wqp(hf, nb2, mt)
                            sb = hf * 1024 + nb2 * 512
                            pqk = pp.tile([128, 512], FP32, tag="pqk", name=f"pqk{hf}{mt}{nb2}")
                            xslab = xh_lo if nb2 == 0 else get_x8(hf, 1)
                            k2s = tuple(range(0, KT, 2))
                            for ik, k2 in enumerate(k2s):
                                wsrc = wqlo if k2 < 16 else wqhi
                                kk = k2 % 16
                                nc.tensor.matmul(
                                    pqk[:],
                                    lhsT=wsrc[:, kk:kk + 2, :],
                                    rhs=xslab[k2 // 8][:, k2 % 8:k2 % 8 + 2, :],
                                    start=(ik == 0),
                                    stop=(ik == len(k2s) - 1),
                                    perf_mode=DR,
                                )
                            qkraw = evict.tile([128, 512], BF16, tag="qkraw",
                                               name=f"qkraw{hf}{mt}{nb2}")
                            nc.scalar.copy(qkraw[:], pqk[:])
                            flush_rope(qkraw, mt, sb)
                            if hf == 0 and nb2 == 1 and mt == 1:
                                # prefetch the second half's lo slab: its WAR
                                # (this half's nb2=0 readers) has just cleared,
                                # so the trigger fires immediately and the 2MB
                                # transfer hides under nb2=1 compute
                                get_x8(1, 0)
                            if hf == 1 and nb2 == 1 and mt == 7:
                                # prefetch the v phase's first x slab quarters
                                load_vx(0, 0, xpoolv, 0)
                            if hf == 1 and nb2 == 1 and mt == 9:
                                # prefetch the v phase's first weight panels so
                                # the qk->v transition has no DMA bubble
                                load_wv(0, 0, 0)
                                load_wv(0, 0, 1)

            # ------ phase 1b: v projection (bf16), straight into SBUF ------
            with ExitStack() as s1b:
                xpool2 = s1b.enter_context(tc.tile_pool(name="xpv2", bufs=1))
                xpool3 = s1b.enter_context(tc.tile_pool(name="xpv3", bufs=1))

                pp = s1b.enter_context(tc.tile_pool(name="ppb", bufs=2, space="PSUM"))

                for hf in range(2):
                    xh_lo = vxh.get((hf, 0)) or load_vx(hf, 0, xpool3, 2)
                    xh_hi = vxh.get((hf, 1)) or load_vx(hf, 1, xpool2, 1)

                    def xh_chunk(k, c0, w, xh_lo=xh_lo, xh_hi=xh_hi):
                        # columns [c0, c0+w) of this half's x, no slab cross
                        quarters = xh_lo if c0 < 512 else xh_hi
                        cc = c0 % 512
                        assert cc + w <= 512
                        return quarters[k // 8][:, k % 8, cc:cc + w]

                    # v projection: v[seq, vdim] natural layout; N=512 panels
                    # split into 4 half-K tiles so DMAs stay 0.5MB
                    for nb in range(2):
                        panels = []
                        for kh in range(2):
                            halves = wvh.get((hf, nb, kh))
                            if halves is None:
                                halves = load_wv(hf, nb, kh)
                            panels.extend(halves)
                        # panels[i] covers ktiles [8i, 8i+8)
                        for mt in range(8):
                            if hf == 0 and nb == 1:
                                # prefetch hf1's slabs and panels under hf0's
                                # nb=1 compute: the lo slab rides its own tag
                                # set (no WAR), the hi slab fires once hf0's
                                # hi readers clear, and the first weight
                                # panels keep the qk..er hf transition fed
                                if mt == 2:
                                    load_vx(1, 0, xpool3, 2)
                                elif mt == 3:
                                    load_wv(1, 0, 0)
                                elif mt == 5:
                                    load_wv(1, 0, 1)
                                elif mt == 6:
                                    load_vx(1, 1, xpool2, 1)
                            pv = pp.tile([128, 512], FP32, tag="pv", name=f"pv{hf}{nb}{mt}")
                            for k in range(KT):
                                nc.tensor.matmul(
                                    pv[:],
                                    lhsT=xh_chunk(k, mt * 128, 128),
                                    rhs=panels[k // 8][:, k % 8, :],
                                    start=(k == 0),
                                    stop=(k == KT - 1),
                                )
                            st = hf * 8 + mt
                            # DVE eviction keeps the ACT queue free so the next
                            # half's x-slab DMA triggers fire immediately
                            nc.vector.tensor_copy(
                                v_all[:, st, nb * 512:(nb + 1) * 512], pv[:])

        # ---------- phases 2+3: attention, then output projection ----------
        with ExitStack() as s2:
            c2 = s2.enter_context(tc.tile_pool(name="c2", bufs=1))
            apool = s2.enter_context(tc.tile_pool(name="ap", bufs=1))
            eppool = s2.enter_context(tc.tile_pool(name="ep", bufs=6))
            sqpool = s2.enter_context(tc.tile_pool(name="sq", bufs=4))
            wpool3 = s2.enter_context(tc.tile_pool(name="wp3", bufs=2))
            s2p = s2.enter_context(ExitStack())
            att_ps = s2p.enter_context(tc.tile_pool(name="attps", bufs=2, space="PSUM"))
            av_ps = s2p.enter_context(tc.tile_pool(name="avps", bufs=2, space="PSUM"))
            l_ps = s2p.enter_context(tc.tile_pool(name="lps", bufs=1, space="PSUM"))

            msk_sb = c2.tile([128, 128], BF16, tag="msk", name="msk_sb")
            nc.sync.dma_start(out=msk_sb[:], in_=msk[:])

            attnT = [apool.tile([128, S], BF16, tag=f"attnT{t}", name=f"attnT{t}")
                     for t in range(8)]

            # one l bank pair reused across all j; garbage rows only ever feed
            # unused reciprocal lanes
            lA = l_ps.tile([128, 512], FP32, tag="lA", name="lA")
            lB = l_ps.tile([128, 512], FP32, tag="lB", name="lB")
            nc.vector.memset(lA[:], 1.0)
            nc.vector.memset(lB[:], 1.0)

            # linv tiles allocated up front: their 128-col reciprocal chunks
            # are emitted ONE PER (j,h) GROUP during the next j's groups, so
            # the 0.85us iterative-divide ops never pile up on the DVE queue
            # ahead of the mask/quad ops the PE pipeline depends on.
            linvs = [
                (apool.tile([128, 512], BF16, tag=f"linvA{j}", name=f"linvA{j}"),
                 apool.tile([128, 512], BF16, tag=f"linvB{j}", name=f"linvB{j}"))
                for j in range(4)
            ]
            lsd = {}

            def emit_recip(jsrc, idx):
                bank = 0 if idx < 4 else 1
                cc = (idx % 4) * 128
                src = lsd[(jsrc, bank)]
                dst = linvs[jsrc][bank]
                # The deferred-emission point (next group's score stretch) IS
                # the DVE idle window, so no priority offset: shifting later
                # would land the recip back among that group's masks/quads.
                with nc.allow_low_precision(reason="bf16 1/l: +1e-3 rel err, single-pass bcast matmul"):
                    nc.vector.reciprocal(dst[:, cc:cc + 128], src[:, cc:cc + 128])

            # l-matmuls of group g are emitted inside group g+1 (after its
            # 4th score emission) so the PE never waits on the DVE adds/masks
            # that produce g's quad tiles.  Each entry is a closure.
            pending_lops = []

            for j in range(4):
                ni = 4 * j + 4
                for h in range(8):
                    lbank = lA if h < 4 else lB
                    hp = (h % 4) * 32
                    pav = av_ps.tile([128, 512], FP32, tag="pav", name=f"pav{j}{h}")

                    eps = {}
                    pair_buf = {}
                    diag_lops = []
                    quad_lops = []

                    def c_lo(i, j=j):
                        r = i - 4 * j
                        return 128 * r if r > 0 else 0

                    def emit_score(i, j=j, h=h):
                        # scores land in 2-bank paired PSUM tiles; consecutive
                        # FULL tiles share ONE exp over [128, 2, 512] (ACT op
                        # count 320 -> 224, and the per-op overhead halves on
                        # the bulk).  Diagonal tiles (ragged c0) keep their own
                        # exp on their sub-slice; the strip [c0, c0+128) gets
                        # the triangular mask.
                        c0 = c_lo(i)
                        slot, sub = i // 2, i % 2
                        if sub == 0:
                            psc2 = att_ps.tile([128, 2, 512], FP32, tag="psc",
                                               name=f"psc{j}{h}{slot}")
                            ep2 = eppool.tile([128, 2, 512], BF16, tag="ep",
                                              name=f"ep{j}{h}{slot}")
                            pair_buf[slot] = (psc2, ep2)
                        psc2, ep2 = pair_buf[slot]
                        nc.tensor.matmul(
                            psc2[:, sub, c0:512],
                            lhsT=qkrot[8 + h][:, i * 128:(i + 1) * 128],
                            rhs=qkrot[h][:, j * 512 + c0:(j + 1) * 512],
                            start=True, stop=True,
                        )
                        if c0 > 0:
                            nc.scalar.activation(ep2[:, sub, c0:512],
                                                 psc2[:, sub, c0:512],
                                                 EXP, scale=SCALE_Q)
                        elif sub == 1:
                            # both subs full-width: one exp over the pair
                            nc.scalar.activation(ep2[:, :, :], psc2[:, :, :],
                                                 EXP, scale=SCALE_Q)
                        elif i == 4 * j:
                            # even full-width tile whose partner is ragged
                            nc.scalar.activation(ep2[:, 0, :], psc2[:, 0, :],
                                                 EXP, scale=SCALE_Q)
                        if i - 4 * j >= 0:
                            # triangular mask on the OTHERWISE-IDLE GpSimd:
                            # keeps the in-order DVE queue (recips, quad adds,
                            # evictions) out of the exp->mask->av critical path
                            nc.gpsimd.tensor_mul(ep2[:, sub, c0:c0 + 128],
                                                 ep2[:, sub, c0:c0 + 128],
                                                 msk_sb[:])
                        eps[i] = (ep2, sub)

                    # l reduction plan: full tiles (i < 4j) are pre-summed in
                    # quads on the DVE (one ones-matmul per 4 tiles); the 4
                    # ragged diagonal tiles go straight to the PE.
                    quad = []   # full-width ep tiles awaiting quad reduction
                    nq_flushed = [0]

                    def flush_quad(j=j, h=h):
                        assert len(quad) == 4
                        nq = nq_flushed[0]
                        q0 = sqpool.tile([128, 512], BF16, tag="q0",
                                         name=f"q0_{j}{h}{nq}")
                        q1 = sqpool.tile([128, 512], BF16, tag="q1",
                                         name=f"q1_{j}{h}{nq}")
                        qq = sqpool.tile([128, 512], BF16, tag="qq",
                                         name=f"qq_{j}{h}{nq}")
                        (at_, as_), (bt_, bs_), (ct_, cs_), (dt_, ds_) = quad
                        nc.vector.tensor_add(q0[:], at_[:, as_, :], bt_[:, bs_, :])
                        nc.vector.tensor_add(q1[:], ct_[:, cs_, :], dt_[:, ds_, :])
                        nc.vector.tensor_add(qq[:], q0[:], q1[:])
                        nq_flushed[0] += 1
                        quad.clear()
                        return qq

                    # the group's l-matmul sequence: 4 ragged diag tiles first
                    # (the ii==4j one is full width and carries start=True),
                    # then the quad matmuls, the last carrying stop=True.
                    def add_diag_lop(ep2, sub, c0, ii, j=j, ni=ni, lbank=lbank, hp=hp):
                        def op():
                            nc.tensor.matmul(
                                lbank[hp:hp + 1, c0:512],
                                lhsT=ones_sb[:, 0:1],
                                rhs=ep2[:, sub, c0:512],
                                start=(ii == 4 * j),
                                stop=(j == 0 and ii == ni - 1),
                                tile_position=(0, hp),
                            )
                        diag_lops.append(op)

                    def add_quad_lop(qq, is_last, lbank=lbank, hp=hp):
                        def op():
                            nc.tensor.matmul(
                                lbank[hp:hp + 1, :],
                                lhsT=ones_sb[:, 0:1],
                                rhs=qq[:],
                                start=False, stop=is_last,
                                tile_position=(0, hp),
                            )
                        quad_lops.append(op)

                    # software-pipeline: scores run 4 tiles ahead of l/av so the
                    # exp+mask latency never stalls the PE
                    LOOKAHEAD = 4
                    for i in range(ni + LOOKAHEAD):
                        if i == 4:
                            # PE has ~1us of this group's scores queued: emit
                            # the PREVIOUS group's l-matmuls now
                            for op in pending_lops:
                                op()
                            pending_lops.clear()
                        if i < ni:
                            emit_score(i)
                        ii = i - LOOKAHEAD
                        if ii < 0:
                            continue
                        ep2, sub = eps.pop(ii)
                        c0 = c_lo(ii)
                        if ii < 4 * j:
                            quad.append((ep2, sub))
                            if len(quad) == 4:
                                qq = flush_quad()
                                # the j-th (last) quad of the group ends the
                                # lbank row's accumulation group
                                add_quad_lop(qq, is_last=(ii == 4 * j - 1))
                        else:
                            add_diag_lop(ep2, sub, c0, ii)
                        nc.tensor.matmul(
                            pav[:, c0:512],
                            lhsT=v_all[:, ii, h * 128:(h + 1) * 128],
                            rhs=ep2[:, sub, c0:512],
                            start=(ii == 0), stop=(ii == ni - 1),
                        )
                    assert not quad
                    # execution order: diags first (ii==4j carries start=True),
                    # then quads (last quad carries stop for j>0)
                    pending_lops = diag_lops + quad_lops

                    # DVE copy: keeps the ScalarE exp-only during attention (no
                    # activation-table thrash between Copy and Exp)
                    nc.vector.tensor_copy(attnT[h][:, j * 512:(j + 1) * 512], pav[:])
                    # evict each l bank right after its LAST writer (lA: h==3,
                    # lB: h==7): deferred together with the l-matmuls
                    if h == 3:
                        def evA(j=j):
                            lsA = apool.tile([128, 512], FP32, tag=f"lsA{j}", name=f"lsA{j}")
                            nc.vector.tensor_copy(lsA[:], lA[:])
                            lsd[(j, 0)] = lsA
                        pending_lops.append(evA)
                    elif h == 7:
                        def evB(j=j):
                            lsB = apool.tile([128, 512], FP32, tag=f"lsB{j}", name=f"lsB{j}")
                            nc.vector.tensor_copy(lsB[:], lB[:])
                            lsd[(j, 1)] = lsB
                        pending_lops.append(evB)
                    # one reciprocal chunk per group, spread so they never
                    # head-of-line-block the DVE.  Emitted as deferred closures
                    # since lsd entries appear one group later now.
                    if j > 0:
                        def rec(j=j, h=h):
                            emit_recip(j - 1, h)
                        pending_lops.append(rec)
                    if j == 3 and h >= 4:
                        def rec2(h=h):
                            emit_recip(3, h - 4)
                        pending_lops.append(rec2)

            # flush the final group's deferred ops; the remaining bank-B
            # reciprocals are interleaved into the bc pass below (after j=1)
            # so they never head-of-line-block the bc normalize muls
            for op in pending_lops:
                op()
            pending_lops.clear()

            # prefetch the first o-proj weight panels under the bc pass
            wo_cache = {}

            def get_wop(nb):
                if nb not in wo_cache:
                    lo = wpool3.tile([128, 4, 512], BF16, tag="woplo",
                                     name=f"woplo{nb}")
                    hi = wpool3.tile([128, 4, 512], BF16, tag="wophi",
                                     name=f"wophi{nb}")
                    nc.scalar.dma_start(out=lo[:, :, :], in_=wo[nb, :, 0:4, :])
                    nc.scalar.dma_start(out=hi[:, :, :], in_=wo[nb, :, 4:8, :])
                    wo_cache[nb] = (lo, hi)
                return wo_cache[nb]

            get_wop(0)

            # normalize attn_outT by 1/l (broadcast 1/l across partitions);
            # bc's PSUM bank comes from a fresh scope so the main loop can run
            # a 4-deep score ring within the 8-bank budget
            s2p.close()
            with ExitStack() as s2n:
                bc_ps = s2n.enter_context(tc.tile_pool(name="bcps", bufs=2, space="PSUM"))
                for j in range(4):
                    if j == 2:
                        # bank-B j=3 recips here: the 16 bc muls already
                        # queued keep the PE fed while these run, and they
                        # finish well before bc(3, h>=4) reads linvB3
                        for idx in range(4, 8):
                            emit_recip(3, idx)
                    for h in range(8):
                        linv = linvs[j][0] if h < 4 else linvs[j][1]
                        hp = (h % 4) * 32
                        bc = bc_ps.tile([128, 512], FP32, tag="bc", name=f"bc{j}{h}")
                        nc.tensor.matmul(
                            bc[:],
                            lhsT=ones_sb[hp:hp + 1, :],
                            rhs=linv[hp:hp + 1, :],
                            start=True, stop=True,
                            tile_position=(hp, 0),
                        )
                        nc.vector.tensor_mul(
                            attnT[h][:, j * 512:(j + 1) * 512],
                            attnT[h][:, j * 512:(j + 1) * 512],
                            bc[:],
                        )

            # ---------- phase 3: output projection --------------------------
            with ExitStack() as s3:
                ev3 = s3.enter_context(tc.tile_pool(name="ev3", bufs=4))
                po_ps = s3.enter_context(tc.tile_pool(name="pops", bufs=2, space="PSUM"))
                for nb in range(8):
                    # wop rides the Scalar HWDGE queue so it is never stuck
                    # behind the output-tile writes
                    woplo, wophi = get_wop(nb)
                    for mt in range(16):
                        po = po_ps.tile([128, 512], FP32, tag="po", name=f"po{nb}{mt}")
                        for k in range(8):
                            wsrc = woplo if k < 4 else wophi
                            nc.tensor.matmul(
                                po[:],
                                lhsT=attnT[k][:, mt * 128:(mt + 1) * 128],
                                rhs=wsrc[:, k % 4, :],
                                start=(k == 0), stop=(k == 7),
                            )
                        if mt == 0 and nb + 1 < 8:
                            get_wop(nb + 1)
                        osb = ev3.tile([128, 512], FP32, tag="osb", name=f"osb{nb}{mt}")
                        nc.scalar.copy(osb[:], po[:])
                        # 32MB of f32 partials: alternate HWDGE queues so
                        # neither saturates and backpressures the evict pool
                        dq = nc.sync if mt % 2 == 0 else nc.scalar
                        dq.dma_start(
                            out=out[mt * 128:(mt + 1) * 128, nb * 512:(nb + 1) * 512],
                            in_=osb[:],
                        )

    nc.finalize()
    return nc


def _rope_tables(pos_row):
    """cos/sinn tables [128, S]: row p uses inv_freq[p % 64]; the 1/(SX*SW)
    fp8 descale for q,k is folded in.  sinn rows 0-63 are NEGATED so the DVE
    half-swap (t2[0:64] = qkraw[64:128] * sinn[0:64]) carries the rotation
    sign without a separate table."""
    inv = 1.0 / (ROPE_BASE ** (np.arange(0, HD, 2, dtype=np.float32) / HD))  # [64]
    inv128 = np.concatenate([inv, inv]).astype(np.float32)                   # [128]
    ang = inv128[:, None] * pos_row[None, :].astype(np.float32)              # [128, S]
    ds = SQK / (SX * SW)
    sgn = np.concatenate([-np.ones(64, np.float32), np.ones(64, np.float32)])
    return ((np.cos(ang) * ds).astype(bf16),
            (np.sin(ang) * ds * sgn[:, None]).astype(bf16))


def _consts():
    # triangular tile mask: msk[p, c] = 1 iff c >= p
    msk = np.triu(np.ones((128, 128), np.float32))
    ones = np.ones((128, 128), np.float32)
    return msk.astype(bf16), ones.astype(bf16)


def prep_in_maps(hidden_states, w_pack, w_o, positions):
    hidden_states = np.asarray(hidden_states, dtype=np.float32)
    w_pack = np.asarray(w_pack, dtype=np.float32)
    w_o = np.asarray(w_o, dtype=np.float32)
    positions = np.asarray(positions)

    msk, ones = _consts()
    in_maps = []
    for c in range(8):
        b, g = divmod(c, 4)
        # All layouts are slab/panel-major with the partition dim outermost
        # under the panel index, so every DMA reads 2-16KB CONTIGUOUS per
        # partition line (strided 128-512B lines measured ~148GB/s/queue).
        xT = np.ascontiguousarray(hidden_states[b].T)                  # [H, S]
        x_np = np.ascontiguousarray(
            xT.astype(bf16).reshape(KT, 128, 2, 2, 512).transpose(2, 3, 1, 0, 4))
        x8_np = np.ascontiguousarray(
            np.clip(xT * SX, -240, 240).astype(f8)
            .reshape(KT, 128, 2, 2, 512).transpose(2, 3, 1, 0, 4))
        qbase = g * 1024
        kbase = H + g * 1024
        vbase = 2 * H + g * 1024
        wqk_np = np.empty((16, 128, KT, 128), f8)
        for mt in range(16):
            base = qbase + 128 * mt if mt < 8 else kbase + 128 * (mt - 8)
            blk = w_pack[base:base + 128, :]                      # [128, H]
            wqk_np[mt] = (np.clip(blk.T * SW, -240, 240).astype(f8)
                          .reshape(KT, 128, 128).transpose(1, 0, 2))
        wv_np = np.empty((4, 128, 16, 512), bf16)
        for nb in range(2):
            blk = w_pack[vbase + 512 * nb: vbase + 512 * (nb + 1), :]  # [512, H]
            arr = blk.T.astype(bf16).reshape(2, 16, 128, 512)          # [kh, kk, p, c]
            wv_np[2 * nb] = arr[0].transpose(1, 0, 2)
            wv_np[2 * nb + 1] = arr[1].transpose(1, 0, 2)
        woT = np.ascontiguousarray(w_o[:, g * 1024:(g + 1) * 1024].T)  # [1024, H]
        wo_np = np.ascontiguousarray(
            woT.reshape(8, 128, 8, 512).transpose(2, 1, 0, 3)
        ).astype(bf16)
        cos_np, sinn_np = _rope_tables(positions[b])
        in_maps.append({
            "x": x_np, "x8": x8_np, "wqk": wqk_np, "wv": wv_np, "wo": wo_np,
            "cos": cos_np, "sinn": sinn_np,
            "msk": msk, "ones": ones,
        })
    return in_maps


def kernel(hidden_states, w_pack, w_o, positions, _run_kwargs=None):
    if "nc" not in _NC_CACHE:
        _NC_CACHE["nc"] = build_nc()
    nc = _NC_CACHE["nc"]
    in_maps = prep_in_maps(hidden_states, w_pack, w_o, positions)
    res = run_bass_kernel_spmd(nc, in_maps, core_ids=list(range(8)),
                               **(_run_kwargs or {}))
    _NC_CACHE["last_result"] = res
    out = np.zeros((B, S, H), np.float32)
    for c in range(8):
        b = c // 4
        out[b] += res.results[c]["out"]
    return out


# revision 43
# speedup vs baseline: 1.0348x; 1.0348x over previous
"""Trainium2 Bass kernel for Baichuan attention (B=2, S=2048, H=4096, 32 heads).

Sharding: 8 cores = 2 (batch) x 4 (head groups of 8 heads), tensor-parallel
mirror of ColumnParallelLinear/RowParallelLinear. Each core computes, for its
batch b and head group g:
    qkT   = (w_pack q,k slice) @ x_b.T        [2048 qkdims, 2048 seq]
            in fp8e4 DoubleRow (x,w scaled by 512 each; descale folded into
            the cos/sin rope tables), rope on the DVE via a signed
            half-swap read (partition-offset APs + sign-folded sin table)
    v     = x_b @ (w_pack v slice).T          [2048 seq, 1024]  (bf16),
            written straight into a persistent SBUF tile (no DRAM roundtrip)
    per head: scoresT tiles -> exp -> causal mask
              l = ones @ eP (softmax denominators; full tiles pre-summed in
              quads on the DVE so the PE does 1/4 of the ones-matmuls).
              All l-matmuls of a group are DEFERRED into the next group's
              emission so the PE never stalls on the DVE adds/masks at a
              group boundary.
              out_hT = sum v eP
    partial = attn_out @ w_o[:, cols].T       [2048, 4096]  (f32)
Host sums the 4 TP partials per batch (row-parallel all-reduce done on host).

DMA layout: two HWDGE queues only (Sync + Scalar engines). Weight panels ride
Sync; bulk x slabs, cos/sin tables and the o-proj weight panels ride Scalar so
neither stream head-of-line-blocks the other. SBUF input tiles are split into
quarter/half tiles so the first consumer matmul waits only on the first
~256-512KB of DMA, not a whole 2-4MB slab (dependency tracking is
tile-granular).

Self-contained: hardcodes all shapes; only needs concourse + numpy + ml_dtypes.
"""
import math
from contextlib import ExitStack

import numpy as np
import ml_dtypes

import concourse.bass as bass
import concourse.mybir as mybir
import concourse.tile as tile
from concourse import bacc
from concourse.bass_utils import run_bass_kernel_spmd

bf16 = ml_dtypes.bfloat16
f8 = ml_dtypes.float8_e4m3
FP32 = mybir.dt.float32
BF16 = mybir.dt.bfloat16
F8E4 = mybir.dt.float8e4
DR = mybir.MatmulPerfMode.DoubleRow

B, S, H = 2, 2048, 4096
NH_TOT, HD = 32, 128
NHL = 8                # heads per core
KT = H // 128          # 32 contraction tiles for the projections
VD = NHL * HD          # 1024 local v dims
SCALE = 1.0 / math.sqrt(HD)
ROPE_BASE = 10000.0
SX = 512.0             # fp8 input scale for x
SW = 512.0             # fp8 input scale for w_pack qk rows
SQK = 32.0             # fp8 storage scale for rotated q,k
SCALE_Q = SCALE / (SQK * SQK)  # exp scale with the x32 qkrot descale folded in

_NC_CACHE = {}


def build_nc():
    nc = bacc.Bacc()
    x = nc.declare_dram_parameter("x", [2, 2, 128, KT, 512], BF16, isOutput=False)
    x8 = nc.declare_dram_parameter("x8", [2, 2, 128, KT, 512], F8E4, isOutput=False)
    wqk = nc.declare_dram_parameter("wqk", [16, 128, KT, 128], F8E4, isOutput=False)
    wv = nc.declare_dram_parameter("wv", [4, 128, 16, 512], BF16, isOutput=False)
    wo = nc.declare_dram_parameter("wo", [8, 128, 8, 512], BF16, isOutput=False)
    cos = nc.declare_dram_parameter("cos", [128, S], BF16, isOutput=False)
    sinn = nc.declare_dram_parameter("sinn", [128, S], BF16, isOutput=False)
    msk = nc.declare_dram_parameter("msk", [128, 128], BF16, isOutput=False)
    ones = nc.declare_dram_parameter("ones", [128, 128], BF16, isOutput=False)
    out = nc.declare_dram_parameter("out", [S, H], FP32, isOutput=True)

    EXP = mybir.ActivationFunctionType.Exp

    with tile.TileContext(nc) as tc, ExitStack() as g:
        glob = g.enter_context(tc.tile_pool(name="glob", bufs=1))

        # qkrot in fp8 (x32 scale, descale folded into the exp scale): the
        # qk-path quantization washes out through the near-uniform softmax,
        # and fp8 halves the dominant SBUF tensor (32KB vs 64KB)
        qkrot = [glob.tile([128, S], F8E4, tag=f"qkrot{t}", name=f"qkrot{t}")
                 for t in range(16)]
        v_all = glob.tile([128, 16, VD], BF16, tag="vall", name="vall")
        ones_sb = glob.tile([128, 128], BF16, tag="ones", name="ones_sb")

        # ---------- phase 1a: qk projection (fp8 DoubleRow) + rope ---------
        with ExitStack() as s1v:
            # v-phase pools live one scope up so their first loads can be
            # emitted mid-qk and prefetch during the qk tail
            # Two alternating pools each for x slabs and wv panels: every
            # buffer is read by exactly one contiguous run of chains, and
            # consecutive groups live in different pools, so each group's
            # DMAs fire at EMISSION (one group of lead) with no cross-hf WAR
            # coupling.  wv panels are reloaded per (xq, nb) group (+8MB DMA,
            # far under the Sync queue's spare bandwidth).
            # parity-0 pools live at s1v scope (their first tenants prefetch
            # during the qk tail); parity-1 pools are appended at s1b entry
            # so phase 1a's SBUF peak stays within budget
            vx_pools = [s1v.enter_context(tc.tile_pool(name="vx0", bufs=1))]
            wv_pools = [s1v.enter_context(tc.tile_pool(name="wvb0", bufs=1))]
            vxh = {}
            wvh = {}

            def load_vx(hf, xq):
                # 4 quarter tiles of [128, 8, 512]; slab pool alternates by xq
                pool = vx_pools[xq]
                quarters = []
                for qt in range(4):
                    qtile = pool.tile([128, 8, 512], BF16, tag=f"vq{qt}",
                                      name=f"vxh{hf}{xq}q{qt}")
                    nc.scalar.dma_start(
                        out=qtile[:, :, :],
                        in_=x[hf, xq, :, qt * 8:(qt + 1) * 8, :],
                    )
                    quarters.append(qtile)
                vxh[(hf, xq)] = quarters
                return quarters

            def load_wv(hf, xq, nb):
                # all four k-panels for column block nb, on the Sync queue;
                # pool alternates per group
                gidx = hf * 4 + xq * 2 + nb
                pool = wv_pools[gidx % 2]
                tiles = []
                for kh in range(2):
                    for hh in range(2):
                        t = pool.tile([128, 8, 512], BF16, tag=f"wv{kh}{hh}",
                                      name=f"wv{hf}{xq}{nb}{kh}{hh}")
                        nc.sync.dma_start(out=t[:, :, :],
                                          in_=wv[2 * nb + kh, :, hh * 8:(hh + 1) * 8, :])
                        tiles.append(t)
                wvh[(hf, xq, nb)] = tiles
                return tiles

            with ExitStack() as s1:
                c1 = s1.enter_context(tc.tile_pool(name="c1", bufs=1))
                xpool = s1.enter_context(tc.tile_pool(name="xp", bufs=1))
                wpool = s1.enter_context(tc.tile_pool(name="wp", bufs=6))
                evict = s1.enter_context(tc.tile_pool(name="ev", bufs=2))
                # bufs=1: the rope chain is DVE-only and the DVE is in-order,
                # so extra buffers cannot add overlap
                rope = s1.enter_context(tc.tile_pool(name="rope", bufs=1))
                pp = s1.enter_context(tc.tile_pool(name="pp", bufs=3, space="PSUM"))

                # one tile per 512-col chunk: rope flush sb only waits its own
                # chunk's DMA (dep tracking is tile-granular)
                cosc = [c1.tile([128, 512], BF16, tag=f"cosc{i}", name=f"cosc{i}")
                        for i in range(4)]
                sinnc = [c1.tile([128, 512], BF16, tag=f"sinnc{i}", name=f"sinnc{i}")
                         for i in range(4)]

                def load_rope_chunk(i, eng):
                    eng.dma_start(out=cosc[i][:], in_=cos[:, i * 512:(i + 1) * 512])
                    eng.dma_start(out=sinnc[i][:], in_=sinn[:, i * 512:(i + 1) * 512])

                wq_cache = {}

                def get_wqp(hf_, nb2_, mt_, hi_engine=None):
                    # two half-panels so the first matmul waits on 256KB only
                    key = (hf_, nb2_, mt_)
                    if key not in wq_cache:
                        lo = wpool.tile([128, 16, 128], F8E4, tag="wqplo",
                                        name=f"wqplo{hf_}{nb2_}{mt_}")
                        hi = wpool.tile([128, 16, 128], F8E4, tag="wqphi",
                                        name=f"wqphi{hf_}{nb2_}{mt_}")
                        nc.sync.dma_start(out=lo[:, :, :],
                                          in_=wqk[mt_, :, 0:16, :])
                        (hi_engine or nc.sync).dma_start(out=hi[:, :, :],
                                                         in_=wqk[mt_, :, 16:32, :])
                        wq_cache[key] = (lo, hi)
                    return wq_cache[key]

                x8_cache = {}

                def get_x8(hf_, xq_, engines=None):
                    # 4 quarter tiles [128, 8, 512] per 512-col slab; x loads
                    # ride the Scalar HWDGE queue in 4-ktile chunks so they
                    # never head-of-line-block the wqp panel stream (Sync).
                    # `engines` overrides the queue per quarter.
                    key = (hf_, xq_)
                    if key not in x8_cache:
                        tag = "xhlo" if xq_ == 0 else "xhhi"
                        quarters = []
                        for qt in range(4):
                            qtile = xpool.tile([128, 8, 512], F8E4,
                                               tag=f"{tag}q{qt}",
                                               name=f"x8{tag[2:]}{hf_}q{qt}")
                            eng = engines[qt] if engines else nc.scalar
                            for c in range(2):
                                eng.dma_start(
                                    out=qtile[:, c * 4:(c + 1) * 4, :],
                                    in_=x8[hf_, xq_, :, qt * 8 + c * 4:qt * 8 + (c + 1) * 4, :],
                                )
                            quarters.append(qtile)
                        x8_cache[key] = quarters
                    return x8_cache[key]

                # --- start-ramp preamble -------------------------------------
                # Neither queue alone can feed tiles 0-5 at PE pace (a panel
                # is 512KB/3.4us = one queue's line rate, and tile 0 needs the
                # whole 2MB lo slab), so split the ramp across BOTH queues.
                # (The Scalar engine's ACT_TABLE_LOAD delays its queue ~3us,
                # so the most-critical first bytes ride Sync.)
                get_wqp(0, 0, 0, hi_engine=nc.scalar)
                get_x8(0, 0, engines=[nc.sync, nc.scalar, nc.sync, nc.sync])
                for mt_pre in range(1, 6):
                    get_wqp(0, 0, mt_pre, hi_engine=nc.scalar)
                load_rope_chunk(0, nc.sync)
                nc.sync.dma_start(out=ones_sb[:], in_=ones[:])
                load_rope_chunk(1, nc.scalar)

                # partition p <- p+64 mod 128, in stream_shuffle's 4-partition
                # group units (32 groups, shift by 16)
                SWAP_MASK = [(gg + 16) % 32 for gg in range(32)]

                def flush_rope(qkraw, mt_p, sb_p):
                    # DVE-only rope: swap(q) is a signed row permutation of
                    # the SAME projection output — STREAM_SHUFFLE rotates the
                    # partitions by 64, and the sign rides the sinn table.
                    t1 = rope.tile([128, 512], BF16, tag="t1", name=f"t1_{mt_p}{sb_p}")
                    t2 = rope.tile([128, 512], BF16, tag="t2", name=f"t2_{mt_p}{sb_p}")
                    t2s = rope.tile([128, 512], BF16, tag="t2s", name=f"t2s_{mt_p}{sb_p}")
                    ci = sb_p // 512
                    nc.vector.stream_shuffle(t2s[:], qkraw[:], mask=SWAP_MASK)
                    nc.vector.tensor_mul(t2[:], t2s[:], sinnc[ci][:])
                    nc.vector.tensor_mul(t1[:], qkraw[:], cosc[ci][:])
                    nc.vector.tensor_add(qkrot[mt_p][:, sb_p:sb_p + 512], t1[:], t2[:])

                for hf in range(2):
                    xh_lo = get_x8(hf, 0)

                    # nb2-outer so each 512-col slab of xh has its last reader
                    # at the end of one sub-phase: the next half's x DMA for
                    # that slab overlaps the other slab's compute.  The hi
                    # slab's DMA is deferred to a mid-nb2=0 hook so the ramp
                    # traffic clears the Scalar queue first.
                    for nb2 in range(2):
                        for mt in range(16):
                            if nb2 == 0 and mt == 8:
                                if hf == 0:
                                    load_rope_chunk(2, nc.scalar)
                                    load_rope_chunk(3, nc.scalar)
                                get_x8(hf, 1)
                            wqlo, wqhi = get_wqp(hf, nb2, mt)
                            sb = hf * 1024 + nb2 * 512
                            pqk = pp.tile([128, 512], FP32, tag="pqk", name=f"pqk{hf}{mt}{nb2}")
                            xslab = xh_lo if nb2 == 0 else get_x8(hf, 1)
                            k2s = tuple(range(0, KT, 2))
                            for ik, k2 in enumerate(k2s):
                                wsrc = wqlo if k2 < 16 else wqhi
                                kk = k2 % 16
                                nc.tensor.matmul(
                                    pqk[:],
                                    lhsT=wsrc[:, kk:kk + 2, :],
                                    rhs=xslab[k2 // 8][:, k2 % 8:k2 % 8 + 2, :],
                                    start=(ik == 0),
                                    stop=(ik == len(k2s) - 1),
                                    perf_mode=DR,
                                )
                            qkraw = evict.tile([128, 512], BF16, tag="qkraw",
                                               name=f"qkraw{hf}{mt}{nb2}")
                            nc.scalar.copy(qkraw[:], pqk[:])
                            flush_rope(qkraw, mt, sb)
                            if hf == 0 and nb2 == 1 and mt == 1:
                                # prefetch the second half's lo slab: its WAR
                                # (this half's nb2=0 readers) has just cleared,
                                # so the trigger fires immediately and the 2MB
                                # transfer hides under nb2=1 compute
                                get_x8(1, 0)
                            if hf == 1 and nb2 == 1 and mt == 7:
                                # prefetch the v phase's first x slab quarters
                                load_vx(0, 0)
                            if hf == 1 and nb2 == 1 and mt == 9:
                                # prefetch the v phase's first weight panels so
                                # the qk->v transition has no DMA bubble
                                load_wv(0, 0, 0)

            # ------ phase 1b: v projection (bf16), straight into SBUF ------
            # Group order per hf: (xq0,nb0) (xq0,nb1) (xq1,nb0) (xq1,nb1);
            # each group = 4 chains over one x slab + one panel set.
            with ExitStack() as s1b:
                pp = s1b.enter_context(tc.tile_pool(name="ppb", bufs=2, space="PSUM"))
                vx_pools.append(s1b.enter_context(tc.tile_pool(name="vx1", bufs=1)))
                wv_pools.append(s1b.enter_context(tc.tile_pool(name="wvb1", bufs=1)))

                groups = [(hf, xq, nb) for hf in range(2)
                          for xq in range(2) for nb in range(2)]
                for gi, (hf, xq, nb) in enumerate(groups):
                    slab = vxh.get((hf, xq)) or load_vx(hf, xq)
                    panels = wvh.get((hf, xq, nb)) or load_wv(hf, xq, nb)
                    for mt4 in range(4):
                        if mt4 == 1 and gi + 1 < len(groups):
                            # next group's panels: fresh pool parity, fires at
                            # emission -> ~25us of lead
                            nhf, nxq, nnb = groups[gi + 1]
                            if (nhf, nxq, nnb) not in wvh:
                                load_wv(nhf, nxq, nnb)
                        if mt4 == 2 and gi % 2 == 0 and gi + 2 < len(groups):
                            # the slab two groups out (other xq parity): its
                            # pool's previous tenant has no readers left after
                            # this group pair, so the DMA fires promptly
                            nhf, nxq, _ = groups[gi + 2]
                            if (nhf, nxq) not in vxh:
                                load_vx(nhf, nxq)
                        mt = xq * 4 + mt4
                        pv = pp.tile([128, 512], FP32, tag="pv", name=f"pv{hf}{nb}{mt}")
                        for k in range(KT):
                            nc.tensor.matmul(
                                pv[:],
                                lhsT=slab[k // 8][:, k % 8, mt4 * 128:(mt4 + 1) * 128],
                                rhs=panels[k // 8][:, k % 8, :],
                                start=(k == 0),
                                stop=(k == KT - 1),
                            )
                        st = hf * 8 + mt
                        # DVE eviction keeps the ACT queue free so the next
                        # slab's DMA triggers fire immediately
                        nc.vector.tensor_copy(
                            v_all[:, st, nb * 512:(nb + 1) * 512], pv[:])

        # ---------- phases 2+3: attention, then output projection ----------
        with ExitStack() as s2:
            c2 = s2.enter_context(tc.tile_pool(name="c2", bufs=1))
            apool = s2.enter_context(tc.tile_pool(name="ap", bufs=1))
            eppool = s2.enter_context(tc.tile_pool(name="ep", bufs=6))
            sqpool = s2.enter_context(tc.tile_pool(name="sq", bufs=4))
            wpool3 = s2.enter_context(tc.tile_pool(name="wp3", bufs=2))
            s2p = s2.enter_context(ExitStack())
            att_ps = s2p.enter_context(tc.tile_pool(name="attps", bufs=2, space="PSUM"))
            av_ps = s2p.enter_context(tc.tile_pool(name="avps", bufs=2, space="PSUM"))
            l_ps = s2p.enter_context(tc.tile_pool(name="lps", bufs=1, space="PSUM"))

            msk_sb = c2.tile([128, 128], BF16, tag="msk", name="msk_sb")
            nc.sync.dma_start(out=msk_sb[:], in_=msk[:])

            attnT = [apool.tile([128, S], BF16, tag=f"attnT{t}", name=f"attnT{t}")
                     for t in range(8)]

            # one l bank pair reused across all j; garbage rows only ever feed
            # unused reciprocal lanes
            lA = l_ps.tile([128, 512], FP32, tag="lA", name="lA")
            lB = l_ps.tile([128, 512], FP32, tag="lB", name="lB")
            nc.vector.memset(lA[:], 1.0)
            nc.vector.memset(lB[:], 1.0)

            # linv tiles allocated up front: their 128-col reciprocal chunks
            # are emitted ONE PER (j,h) GROUP during the next j's groups, so
            # the 0.85us iterative-divide ops never pile up on the DVE queue
            # ahead of the mask/quad ops the PE pipeline depends on.
            linvs = [
                (apool.tile([128, 512], BF16, tag=f"linvA{j}", name=f"linvA{j}"),
                 apool.tile([128, 512], BF16, tag=f"linvB{j}", name=f"linvB{j}"))
                for j in range(4)
            ]
            lsd = {}

            def emit_recip(jsrc, idx):
                bank = 0 if idx < 4 else 1
                cc = (idx % 4) * 128
                src = lsd[(jsrc, bank)]
                dst = linvs[jsrc][bank]
                # The deferred-emission point (next group's score stretch) IS
                # the DVE idle window, so no priority offset: shifting later
                # would land the recip back among that group's masks/quads.
                with nc.allow_low_precision(reason="bf16 1/l: +1e-3 rel err, single-pass bcast matmul"):
                    nc.vector.reciprocal(dst[:, cc:cc + 128], src[:, cc:cc + 128])

            # l-matmuls of group g are emitted inside group g+1 (after its
            # 4th score emission) so the PE never waits on the DVE adds/masks
            # that produce g's quad tiles.  Each entry is a closure.
            pending_lops = []

            for j in range(4):
                ni = 4 * j + 4
                for h in range(8):
                    lbank = lA if h < 4 else lB
                    hp = (h % 4) * 32
                    pav = av_ps.tile([128, 512], FP32, tag="pav", name=f"pav{j}{h}")

                    eps = {}
                    pair_buf = {}
                    diag_lops = []
                    quad_lops = []

                    def c_lo(i, j=j):
                        r = i - 4 * j
                        return 128 * r if r > 0 else 0

                    def emit_score(i, j=j, h=h):
                        # scores land in 2-bank paired PSUM tiles; consecutive
                        # FULL tiles share ONE exp over [128, 2, 512] (ACT op
                        # count 320 -> 224, and the per-op overhead halves on
                        # the bulk).  Diagonal tiles (ragged c0) keep their own
                        # exp on their sub-slice; the strip [c0, c0+128) gets
                        # the triangular mask.
                        c0 = c_lo(i)
                        slot, sub = i // 2, i % 2
                        if sub == 0:
                            psc2 = att_ps.tile([128, 2, 512], FP32, tag="psc",
                                               name=f"psc{j}{h}{slot}")
                            ep2 = eppool.tile([128, 2, 512], BF16, tag="ep",
                                              name=f"ep{j}{h}{slot}")
                            pair_buf[slot] = (psc2, ep2)
                        psc2, ep2 = pair_buf[slot]
                        nc.tensor.matmul(
                            psc2[:, sub, c0:512],
                            lhsT=qkrot[8 + h][:, i * 128:(i + 1) * 128],
                            rhs=qkrot[h][:, j * 512 + c0:(j + 1) * 512],
                            start=True, stop=True,
                        )
                        if c0 > 0:
                            nc.scalar.activation(ep2[:, sub, c0:512],
                                                 psc2[:, sub, c0:512],
                                                 EXP, scale=SCALE_Q)
                        elif sub == 1:
                            # both subs full-width: one exp over the pair
                            nc.scalar.activation(ep2[:, :, :], psc2[:, :, :],
                                                 EXP, scale=SCALE_Q)
                        elif i == 4 * j:
                            # even full-width tile whose partner is ragged
                            nc.scalar.activation(ep2[:, 0, :], psc2[:, 0, :],
                                                 EXP, scale=SCALE_Q)
                        if i - 4 * j >= 0:
                            # triangular mask on the OTHERWISE-IDLE GpSimd:
                            # keeps the in-order DVE queue (recips, quad adds,
                            # evictions) out of the exp->mask->av critical path
                            nc.gpsimd.tensor_mul(ep2[:, sub, c0:c0 + 128],
                                                 ep2[:, sub, c0:c0 + 128],
                                                 msk_sb[:])
                        eps[i] = (ep2, sub)

                    # l reduction plan: full tiles (i < 4j) are pre-summed in
                    # quads on the DVE (one ones-matmul per 4 tiles); the 4
                    # ragged diagonal tiles go straight to the PE.
                    quad = []   # full-width ep tiles awaiting quad reduction
                    nq_flushed = [0]

                    def flush_quad(j=j, h=h):
                        assert len(quad) == 4
                        nq = nq_flushed[0]
                        q0 = sqpool.tile([128, 512], BF16, tag="q0",
                                         name=f"q0_{j}{h}{nq}")
                        q1 = sqpool.tile([128, 512], BF16, tag="q1",
                                         name=f"q1_{j}{h}{nq}")
                        qq = sqpool.tile([128, 512], BF16, tag="qq",
                                         name=f"qq_{j}{h}{nq}")
                        (at_, as_), (bt_, bs_), (ct_, cs_), (dt_, ds_) = quad
                        nc.vector.tensor_add(q0[:], at_[:, as_, :], bt_[:, bs_, :])
                        nc.vector.tensor_add(q1[:], ct_[:, cs_, :], dt_[:, ds_, :])
                        nc.vector.tensor_add(qq[:], q0[:], q1[:])
                        nq_flushed[0] += 1
                        quad.clear()
                        return qq

                    # the group's l-matmul sequence: 4 ragged diag tiles first
                    # (the ii==4j one is full width and carries start=True),
                    # then the quad matmuls, the last carrying stop=True.
                    def add_diag_lop(ep2, sub, c0, ii, j=j, ni=ni, lbank=lbank, hp=hp):
                        def op():
                            nc.tensor.matmul(
                                lbank[hp:hp + 1, c0:512],
                                lhsT=ones_sb[:, 0:1],
                                rhs=ep2[:, sub, c0:512],
                                start=(ii == 4 * j),
                                stop=(j == 0 and ii == ni - 1),
                                tile_position=(0, hp),
                            )
                        diag_lops.append(op)

                    def add_quad_lop(qq, is_last, lbank=lbank, hp=hp):
                        def op():
                            nc.tensor.matmul(
                                lbank[hp:hp + 1, :],
                                lhsT=ones_sb[:, 0:1],
                                rhs=qq[:],
                                start=False, stop=is_last,
                                tile_position=(0, hp),
                            )
                        quad_lops.append(op)

                    # software-pipeline: scores run 4 tiles ahead of l/av so the
                    # exp+mask latency never stalls the PE
                    LOOKAHEAD = 4
                    for i in range(ni + LOOKAHEAD):
                        if i == 4:
                            # PE has ~1us of this group's scores queued: emit
                            # the PREVIOUS group's l-matmuls now
                            for op in pending_lops:
                                op()
                            pending_lops.clear()
                        if i < ni:
                            emit_score(i)
                        ii = i - LOOKAHEAD
                        if ii < 0:
                            continue
                        ep2, sub = eps.pop(ii)
                        c0 = c_lo(ii)
                        if ii < 4 * j:
                            quad.append((ep2, sub))
                            if len(quad) == 4:
                                qq = flush_quad()
                                # the j-th (last) quad of the group ends the
                                # lbank row's accumulation group
                                add_quad_lop(qq, is_last=(ii == 4 * j - 1))
                        else:
                            add_diag_lop(ep2, sub, c0, ii)
                        nc.tensor.matmul(
                            pav[:, c0:512],
                            lhsT=v_all[:, ii, h * 128:(h + 1) * 128],
                            rhs=ep2[:, sub, c0:512],
                            start=(ii == 0), stop=(ii == ni - 1),
                        )
                    assert not quad
                    # execution order: diags first (ii==4j carries start=True),
                    # then quads (last quad carries stop for j>0)
                    pending_lops = diag_lops + quad_lops

                    # DVE copy: keeps the ScalarE exp-only during attention (no
                    # activation-table thrash between Copy and Exp)
                    nc.vector.tensor_copy(attnT[h][:, j * 512:(j + 1) * 512], pav[:])
                    # evict each l bank right after its LAST writer (lA: h==3,
                    # lB: h==7): deferred together with the l-matmuls
                    if h == 3:
                        def evA(j=j):
                            lsA = apool.tile([128, 512], FP32, tag=f"lsA{j}", name=f"lsA{j}")
                            nc.vector.tensor_copy(lsA[:], lA[:])
                            lsd[(j, 0)] = lsA
                        pending_lops.append(evA)
                    elif h == 7:
                        def evB(j=j):
                            lsB = apool.tile([128, 512], FP32, tag=f"lsB{j}", name=f"lsB{j}")
                            nc.vector.tensor_copy(lsB[:], lB[:])
                            lsd[(j, 1)] = lsB
                        pending_lops.append(evB)
                    # one reciprocal chunk per group, spread so they never
                    # head-of-line-block the DVE.  Emitted as deferred closures
                    # since lsd entries appear one group later now.
                    if j > 0:
                        def rec(j=j, h=h):
                            emit_recip(j - 1, h)
                        pending_lops.append(rec)
                    if j == 3 and h >= 4:
                        def rec2(h=h):
                            emit_recip(3, h - 4)
                        pending_lops.append(rec2)

            # flush the final group's deferred ops; the remaining bank-B
            # reciprocals are interleaved into the bc pass below (after j=1)
            # so they never head-of-line-block the bc normalize muls
            for op in pending_lops:
                op()
            pending_lops.clear()

            # prefetch the first o-proj weight panels under the bc pass
            wo_cache = {}

            def get_wop(nb):
                if nb not in wo_cache:
                    lo = wpool3.tile([128, 4, 512], BF16, tag="woplo",
                                     name=f"woplo{nb}")
                    hi = wpool3.tile([128, 4, 512], BF16, tag="wophi",
                                     name=f"wophi{nb}")
                    nc.scalar.dma_start(out=lo[:, :, :], in_=wo[nb, :, 0:4, :])
                    nc.scalar.dma_start(out=hi[:, :, :], in_=wo[nb, :, 4:8, :])
                    wo_cache[nb] = (lo, hi)
                return wo_cache[nb]

            get_wop(0)

            # normalize attn_outT by 1/l (broadcast 1/l across partitions);
            # bc's PSUM bank comes from a fresh scope so the main loop can run
            # a 4-deep score ring within the 8-bank budget
            s2p.close()
            with ExitStack() as s2n:
                bc_ps = s2n.enter_context(tc.tile_pool(name="bcps", bufs=2, space="PSUM"))
                for j in range(4):
                    if j == 1:
                        # bank-B j=3 recips here: the 8 bc muls already
                        # queued keep the PE fed while these run, and they
                        # finish well before bc(3, h>=4) reads linvB3
                        for idx in range(4, 8):
                            emit_recip(3, idx)
                    for h in range(8):
                        linv = linvs[j][0] if h < 4 else linvs[j][1]
                        hp = (h % 4) * 32
                        bc = bc_ps.tile([128, 512], FP32, tag="bc", name=f"bc{j}{h}")
                        nc.tensor.matmul(
                            bc[:],
                            lhsT=ones_sb[hp:hp + 1, :],
                            rhs=linv[hp:hp + 1, :],
                            start=True, stop=True,
                            tile_position=(hp, 0),
                        )
                        nc.vector.tensor_mul(
                            attnT[h][:, j * 512:(j + 1) * 512],
                            attnT[h][:, j * 512:(j + 1) * 512],
                            bc[:],
                        )

            # ---------- phase 3: output projection --------------------------
            with ExitStack() as s3:
                ev3 = s3.enter_context(tc.tile_pool(name="ev3", bufs=4))
                po_ps = s3.enter_context(tc.tile_pool(name="pops", bufs=2, space="PSUM"))
                for nb in range(8):
                    # wop rides the Scalar HWDGE queue so it is never stuck
                    # behind the output-tile writes
                    woplo, wophi = get_wop(nb)
                    for mt in range(16):
                        po = po_ps.tile([128, 512], FP32, tag="po", name=f"po{nb}{mt}")
                        for k in range(8):
                            wsrc = woplo if k < 4 else wophi
                            nc.tensor.matmul(
                                po[:],
                                lhsT=attnT[k][:, mt * 128:(mt + 1) * 128],
                                rhs=wsrc[:, k % 4, :],
                                start=(k == 0), stop=(k == 7),
                            )
                        if mt == 0 and nb + 1 < 8:
                            get_wop(nb + 1)
                        osb = ev3.tile([128, 512], FP32, tag="osb", name=f"osb{nb}{mt}")
                        nc.scalar.copy(osb[:], po[:])
                        # 32MB of f32 partials: alternate HWDGE queues so
                        # neither saturates and backpressures the evict pool
                        dq = nc.sync if mt % 2 == 0 else nc.scalar
                        dq.dma_start(
                            out=out[mt * 128:(mt + 1) * 128, nb * 512:(nb + 1) * 512],
                            in_=osb[:],
                        )

    nc.finalize()
    return nc


def _rope_tables(pos_row):
    """cos/sinn tables [128, S]: row p uses inv_freq[p % 64]; the 1/(SX*SW)
    fp8 descale for q,k is folded in.  sinn rows 0-63 are NEGATED so the DVE
    half-swap (t2[0:64] = qkraw[64:128] * sinn[0:64]) carries the rotation
    sign without a separate table."""
    inv = 1.0 / (ROPE_BASE ** (np.arange(0, HD, 2, dtype=np.float32) / HD))  # [64]
    inv128 = np.concatenate([inv, inv]).astype(np.float32)                   # [128]
    ang = inv128[:, None] * pos_row[None, :].astype(np.float32)              # [128, S]
    ds = SQK / (SX * SW)
    sgn = np.concatenate([-np.ones(64, np.float32), np.ones(64, np.float32)])
    return ((np.cos(ang) * ds).astype(bf16),
            (np.sin(ang) * ds * sgn[:, None]).astype(bf16))


def _consts():
    # triangular tile mask: msk[p, c] = 1 iff c >= p
    msk = np.triu(np.ones((128, 128), np.float32))
    ones = np.ones((128, 128), np.float32)
    return msk.astype(bf16), ones.astype(bf16)


def prep_in_maps(hidden_states, w_pack, w_o, positions):
    hidden_states = np.asarray(hidden_states, dtype=np.float32)
    w_pack = np.asarray(w_pack, dtype=np.float32)
    w_o = np.asarray(w_o, dtype=np.float32)
    positions = np.asarray(positions)

    msk, ones = _consts()
    in_maps = []
    for c in range(8):
        b, g = divmod(c, 4)
        # All layouts are slab/panel-major with the partition dim outermost
        # under the panel index, so every DMA reads 2-16KB CONTIGUOUS per
        # partition line (strided 128-512B lines measured ~148GB/s/queue).
        xT = np.ascontiguousarray(hidden_states[b].T)                  # [H, S]
        x_np = np.ascontiguousarray(
            xT.astype(bf16).reshape(KT, 128, 2, 2, 512).transpose(2, 3, 1, 0, 4))
        x8_np = np.ascontiguousarray(
            np.clip(xT * SX, -240, 240).astype(f8)
            .reshape(KT, 128, 2, 2, 512).transpose(2, 3, 1, 0, 4))
        qbase = g * 1024
        kbase = H + g * 1024
        vbase = 2 * H + g * 1024
        wqk_np = np.empty((16, 128, KT, 128), f8)
        for mt in range(16):
            base = qbase + 128 * mt if mt < 8 else kbase + 128 * (mt - 8)
            blk = w_pack[base:base + 128, :]                      # [128, H]
            wqk_np[mt] = (np.clip(blk.T * SW, -240, 240).astype(f8)
                          .reshape(KT, 128, 128).transpose(1, 0, 2))
        wv_np = np.empty((4, 128, 16, 512), bf16)
        for nb in range(2):
            blk = w_pack[vbase + 512 * nb: vbase + 512 * (nb + 1), :]  # [512, H]
            arr = blk.T.astype(bf16).reshape(2, 16, 128, 512)          # [kh, kk, p, c]
            wv_np[2 * nb] = arr[0].transpose(1, 0, 2)
            wv_np[2 * nb + 1] = arr[1].transpose(1, 0, 2)
        woT = np.ascontiguousarray(w_o[:, g * 1024:(g + 1) * 1024].T)  # [1024, H]
        wo_np = np.ascontiguousarray(
            woT.reshape(8, 128, 8, 512).transpose(2, 1, 0, 3)
        ).astype(bf16)
        cos_np, sinn_np = _rope_tables(positions[b])
        in_maps.append({
            "x": x_np, "x8": x8_np, "wqk": wqk_np, "wv": wv_np, "wo": wo_np,
            "cos": cos_np, "sinn": sinn_np,
            "msk": msk, "ones": ones,
        })
    return in_maps


def kernel(hidden_states, w_pack, w_o, positions, _run_kwargs=None):
    if "nc" not in _NC_CACHE:
        _NC_CACHE["nc"] = build_nc()
    nc = _NC_CACHE["nc"]
    in_maps = prep_in_maps(hidden_states, w_pack, w_o, positions)
    res = run_bass_kernel_spmd(nc, in_maps, core_ids=list(range(8)),
                               **(_run_kwargs or {}))
    _NC_CACHE["last_result"] = res
    out = np.zeros((B, S, H), np.float32)
    for c in range(8):
        b = c // 4
        out[b] += res.results[c]["out"]
    return out


# revision 45
# speedup vs baseline: 1.0410x; 1.0060x over previous
"""Trainium2 Bass kernel for Baichuan attention (B=2, S=2048, H=4096, 32 heads).

Sharding: 8 cores = 2 (batch) x 4 (head groups of 8 heads), tensor-parallel
mirror of ColumnParallelLinear/RowParallelLinear. Each core computes, for its
batch b and head group g:
    qkT   = (w_pack q,k slice) @ x_b.T        [2048 qkdims, 2048 seq]
            in fp8e4 DoubleRow (x,w scaled by 512 each; descale folded into
            the cos/sin rope tables), rope on the DVE via a signed
            half-swap read (partition-offset APs + sign-folded sin table)
    v     = x_b @ (w_pack v slice).T          [2048 seq, 1024]  (bf16),
            written straight into a persistent SBUF tile (no DRAM roundtrip)
    per head: scoresT tiles -> exp -> causal mask
              l = ones @ eP (softmax denominators; full tiles pre-summed in
              quads on the DVE so the PE does 1/4 of the ones-matmuls).
              All l-matmuls of a group are DEFERRED into the next group's
              emission so the PE never stalls on the DVE adds/masks at a
              group boundary.
              out_hT = sum v eP
    partial = attn_out @ w_o[:, cols].T       [2048, 4096]  (f32)
Host sums the 4 TP partials per batch (row-parallel all-reduce done on host).

DMA layout: two HWDGE queues only (Sync + Scalar engines). Weight panels ride
Sync; bulk x slabs, cos/sin tables and the o-proj weight panels ride Scalar so
neither stream head-of-line-blocks the other. SBUF input tiles are split into
quarter/half tiles so the first consumer matmul waits only on the first
~256-512KB of DMA, not a whole 2-4MB slab (dependency tracking is
tile-granular).

Self-contained: hardcodes all shapes; only needs concourse + numpy + ml_dtypes.
"""
import math
from contextlib import ExitStack

import numpy as np
import ml_dtypes

import concourse.bass as bass
import concourse.mybir as mybir
import concourse.tile as tile
from concourse import bacc
from concourse.bass_utils import run_bass_kernel_spmd

bf16 = ml_dtypes.bfloat16
f8 = ml_dtypes.float8_e4m3
FP32 = mybir.dt.float32
BF16 = mybir.dt.bfloat16
F8E4 = mybir.dt.float8e4
DR = mybir.MatmulPerfMode.DoubleRow

B, S, H = 2, 2048, 4096
NH_TOT, HD = 32, 128
NHL = 8                # heads per core
KT = H // 128          # 32 contraction tiles for the projections
VD = NHL * HD          # 1024 local v dims
SCALE = 1.0 / math.sqrt(HD)
ROPE_BASE = 10000.0
SX = 512.0             # fp8 input scale for x
SW = 512.0             # fp8 input scale for w_pack qk rows
SQK = 32.0             # fp8 storage scale for rotated q,k
SCALE_Q = SCALE / (SQK * SQK)  # exp scale with the x32 qkrot descale folded in

_NC_CACHE = {}


def build_nc():
    nc = bacc.Bacc()
    x = nc.declare_dram_parameter("x", [2, 2, 128, KT, 512], BF16, isOutput=False)
    x8 = nc.declare_dram_parameter("x8", [2, 2, 128, KT, 512], F8E4, isOutput=False)
    wqk = nc.declare_dram_parameter("wqk", [16, 128, KT, 128], F8E4, isOutput=False)
    wv = nc.declare_dram_parameter("wv", [4, 128, 16, 512], BF16, isOutput=False)
    wo = nc.declare_dram_parameter("wo", [8, 128, 8, 512], BF16, isOutput=False)
    cos = nc.declare_dram_parameter("cos", [128, S], BF16, isOutput=False)
    sinn = nc.declare_dram_parameter("sinn", [128, S], BF16, isOutput=False)
    msk = nc.declare_dram_parameter("msk", [128, 128], BF16, isOutput=False)
    ones = nc.declare_dram_parameter("ones", [128, 128], BF16, isOutput=False)
    out = nc.declare_dram_parameter("out", [S, H], FP32, isOutput=True)

    EXP = mybir.ActivationFunctionType.Exp

    with tile.TileContext(nc) as tc, ExitStack() as g:
        glob = g.enter_context(tc.tile_pool(name="glob", bufs=1))

        # qkrot in fp8 (x32 scale, descale folded into the exp scale): the
        # qk-path quantization washes out through the near-uniform softmax,
        # and fp8 halves the dominant SBUF tensor (32KB vs 64KB)
        qkrot = [glob.tile([128, S], F8E4, tag=f"qkrot{t}", name=f"qkrot{t}")
                 for t in range(16)]
        v_all = glob.tile([128, 16, VD], BF16, tag="vall", name="vall")
        ones_sb = glob.tile([128, 128], BF16, tag="ones", name="ones_sb")

        # ---------- phase 1a: qk projection (fp8 DoubleRow) + rope ---------
        with ExitStack() as s1v:
            # v-phase pools live one scope up so their first loads can be
            # emitted mid-qk and prefetch during the qk tail
            # Two alternating pools each for x slabs and wv panels: every
            # buffer is read by exactly one contiguous run of chains, and
            # consecutive groups live in different pools, so each group's
            # DMAs fire at EMISSION (one group of lead) with no cross-hf WAR
            # coupling.  wv panels are reloaded per (xq, nb) group (+8MB DMA,
            # far under the Sync queue's spare bandwidth).
            # parity-0 pools live at s1v scope (their first tenants prefetch
            # during the qk tail); parity-1 pools are appended at s1b entry
            # so phase 1a's SBUF peak stays within budget
            vx_pools = [s1v.enter_context(tc.tile_pool(name="vx0", bufs=1))]
            wv_pools = [s1v.enter_context(tc.tile_pool(name="wvb0", bufs=1))]
            vxh = {}
            wvh = {}

            def load_vx(hf, xq):
                # 4 quarter tiles of [128, 8, 512]; slab pool alternates by xq
                pool = vx_pools[xq]
                quarters = []
                for qt in range(4):
                    qtile = pool.tile([128, 8, 512], BF16, tag=f"vq{qt}",
                                      name=f"vxh{hf}{xq}q{qt}")
                    nc.scalar.dma_start(
                        out=qtile[:, :, :],
                        in_=x[hf, xq, :, qt * 8:(qt + 1) * 8, :],
                    )
                    quarters.append(qtile)
                vxh[(hf, xq)] = quarters
                return quarters

            def load_wv(hf, xq, nb):
                # all four k-panels for column block nb, on the Sync queue;
                # pool alternates per group
                gidx = hf * 4 + xq * 2 + nb
                pool = wv_pools[gidx % 2]
                tiles = []
                for kh in range(2):
                    for hh in range(2):
                        t = pool.tile([128, 8, 512], BF16, tag=f"wv{kh}{hh}",
                                      name=f"wv{hf}{xq}{nb}{kh}{hh}")
                        nc.sync.dma_start(out=t[:, :, :],
                                          in_=wv[2 * nb + kh, :, hh * 8:(hh + 1) * 8, :])
                        tiles.append(t)
                wvh[(hf, xq, nb)] = tiles
                return tiles

            with ExitStack() as s1:
                c1 = s1.enter_context(tc.tile_pool(name="c1", bufs=1))
                xpool = s1.enter_context(tc.tile_pool(name="xp", bufs=1))
                wpool = s1.enter_context(tc.tile_pool(name="wp", bufs=6))
                evict = s1.enter_context(tc.tile_pool(name="ev", bufs=2))
                # bufs=1: the rope chain is DVE-only and the DVE is in-order,
                # so extra buffers cannot add overlap
                rope = s1.enter_context(tc.tile_pool(name="rope", bufs=1))
                pp = s1.enter_context(tc.tile_pool(name="pp", bufs=3, space="PSUM"))

                # one tile per 512-col chunk: rope flush sb only waits its own
                # chunk's DMA (dep tracking is tile-granular)
                cosc = [c1.tile([128, 512], BF16, tag=f"cosc{i}", name=f"cosc{i}")
                        for i in range(4)]
                sinnc = [c1.tile([128, 512], BF16, tag=f"sinnc{i}", name=f"sinnc{i}")
                         for i in range(4)]

                def load_rope_chunk(i, eng):
                    eng.dma_start(out=cosc[i][:], in_=cos[:, i * 512:(i + 1) * 512])
                    eng.dma_start(out=sinnc[i][:], in_=sinn[:, i * 512:(i + 1) * 512])

                wq_cache = {}

                def get_wqp(hf_, nb2_, mt_, hi_engine=None):
                    # two half-panels so the first matmul waits on 256KB only
                    key = (hf_, nb2_, mt_)
                    if key not in wq_cache:
                        lo = wpool.tile([128, 16, 128], F8E4, tag="wqplo",
                                        name=f"wqplo{hf_}{nb2_}{mt_}")
                        hi = wpool.tile([128, 16, 128], F8E4, tag="wqphi",
                                        name=f"wqphi{hf_}{nb2_}{mt_}")
                        nc.sync.dma_start(out=lo[:, :, :],
                                          in_=wqk[mt_, :, 0:16, :])
                        (hi_engine or nc.sync).dma_start(out=hi[:, :, :],
                                                         in_=wqk[mt_, :, 16:32, :])
                        wq_cache[key] = (lo, hi)
                    return wq_cache[key]

                x8_cache = {}

                def get_x8(hf_, xq_, engines=None):
                    # 4 quarter tiles [128, 8, 512] per 512-col slab; x loads
                    # ride the Scalar HWDGE queue in 4-ktile chunks so they
                    # never head-of-line-block the wqp panel stream (Sync).
                    # `engines` overrides the queue per quarter.
                    key = (hf_, xq_)
                    if key not in x8_cache:
                        tag = "xhlo" if xq_ == 0 else "xhhi"
                        quarters = []
                        for qt in range(4):
                            qtile = xpool.tile([128, 8, 512], F8E4,
                                               tag=f"{tag}q{qt}",
                                               name=f"x8{tag[2:]}{hf_}q{qt}")
                            eng = engines[qt] if engines else nc.scalar
                            for c in range(2):
                                eng.dma_start(
                                    out=qtile[:, c * 4:(c + 1) * 4, :],
                                    in_=x8[hf_, xq_, :, qt * 8 + c * 4:qt * 8 + (c + 1) * 4, :],
                                )
                            quarters.append(qtile)
                        x8_cache[key] = quarters
                    return x8_cache[key]

                # --- start-ramp preamble -------------------------------------
                # Neither queue alone can feed tiles 0-5 at PE pace (a panel
                # is 512KB/3.4us = one queue's line rate, and tile 0 needs the
                # whole 2MB lo slab), so split the ramp across BOTH queues.
                # (The Scalar engine's ACT_TABLE_LOAD delays its queue ~3us,
                # so the most-critical first bytes ride Sync.)
                get_wqp(0, 0, 0, hi_engine=nc.scalar)
                get_x8(0, 0, engines=[nc.sync, nc.scalar, nc.sync, nc.sync])
                for mt_pre in range(1, 6):
                    get_wqp(0, 0, mt_pre, hi_engine=nc.scalar)
                load_rope_chunk(0, nc.sync)
                nc.sync.dma_start(out=ones_sb[:], in_=ones[:])
                load_rope_chunk(1, nc.scalar)

                # partition p <- p+64 mod 128, in stream_shuffle's 4-partition
                # group units (32 groups, shift by 16)
                SWAP_MASK = [(gg + 16) % 32 for gg in range(32)]

                def flush_rope(qkraw, mt_p, sb_p):
                    # DVE-only rope: swap(q) is a signed row permutation of
                    # the SAME projection output — STREAM_SHUFFLE rotates the
                    # partitions by 64, and the sign rides the sinn table.
                    t1 = rope.tile([128, 512], BF16, tag="t1", name=f"t1_{mt_p}{sb_p}")
                    t2 = rope.tile([128, 512], BF16, tag="t2", name=f"t2_{mt_p}{sb_p}")
                    t2s = rope.tile([128, 512], BF16, tag="t2s", name=f"t2s_{mt_p}{sb_p}")
                    ci = sb_p // 512
                    nc.vector.stream_shuffle(t2s[:], qkraw[:], mask=SWAP_MASK)
                    nc.vector.tensor_mul(t2[:], t2s[:], sinnc[ci][:])
                    nc.vector.tensor_mul(t1[:], qkraw[:], cosc[ci][:])
                    nc.vector.tensor_add(qkrot[mt_p][:, sb_p:sb_p + 512], t1[:], t2[:])

                for hf in range(2):
                    xh_lo = get_x8(hf, 0)

                    # nb2-outer so each 512-col slab of xh has its last reader
                    # at the end of one sub-phase: the next half's x DMA for
                    # that slab overlaps the other slab's compute.  The hi
                    # slab's DMA is deferred to a mid-nb2=0 hook so the ramp
                    # traffic clears the Scalar queue first.
                    for nb2 in range(2):
                        for mt in range(16):
                            if nb2 == 0 and mt == 8:
                                if hf == 0:
                                    load_rope_chunk(2, nc.scalar)
                                    load_rope_chunk(3, nc.scalar)
                                get_x8(hf, 1)
                            wqlo, wqhi = get_wqp(hf, nb2, mt)
                            sb = hf * 1024 + nb2 * 512
                            pqk = pp.tile([128, 512], FP32, tag="pqk", name=f"pqk{hf}{mt}{nb2}")
                            xslab = xh_lo if nb2 == 0 else get_x8(hf, 1)
                            k2s = tuple(range(0, KT, 2))
                            for ik, k2 in enumerate(k2s):
                                wsrc = wqlo if k2 < 16 else wqhi
                                kk = k2 % 16
                                nc.tensor.matmul(
                                    pqk[:],
                                    lhsT=wsrc[:, kk:kk + 2, :],
                                    rhs=xslab[k2 // 8][:, k2 % 8:k2 % 8 + 2, :],
                                    start=(ik == 0),
                                    stop=(ik == len(k2s) - 1),
                                    perf_mode=DR,
                                )
                            qkraw = evict.tile([128, 512], BF16, tag="qkraw",
                                               name=f"qkraw{hf}{mt}{nb2}")
                            nc.scalar.copy(qkraw[:], pqk[:])
                            flush_rope(qkraw, mt, sb)
                            if hf == 0 and nb2 == 1 and mt == 1:
                                # prefetch the second half's lo slab: its WAR
                                # (this half's nb2=0 readers) has just cleared,
                                # so the trigger fires immediately and the 2MB
                                # transfer hides under nb2=1 compute
                                get_x8(1, 0)
                            if hf == 1 and nb2 == 1 and mt == 7:
                                # prefetch the v phase's first x slab quarters
                                load_vx(0, 0)
                            if hf == 1 and nb2 == 1 and mt == 9:
                                # prefetch the v phase's first weight panels so
                                # the qk->v transition has no DMA bubble
                                load_wv(0, 0, 0)

            # ------ phase 1b: v projection (bf16), straight into SBUF ------
            # Group order per hf: (xq0,nb0) (xq0,nb1) (xq1,nb0) (xq1,nb1);
            # each group = 4 chains over one x slab + one panel set.
            with ExitStack() as s1b:
                pp = s1b.enter_context(tc.tile_pool(name="ppb", bufs=2, space="PSUM"))
                vx_pools.append(s1b.enter_context(tc.tile_pool(name="vx1", bufs=1)))
                wv_pools.append(s1b.enter_context(tc.tile_pool(name="wvb1", bufs=1)))

                groups = [(hf, xq, nb) for hf in range(2)
                          for xq in range(2) for nb in range(2)]
                for gi, (hf, xq, nb) in enumerate(groups):
                    slab = vxh.get((hf, xq)) or load_vx(hf, xq)
                    panels = wvh.get((hf, xq, nb)) or load_wv(hf, xq, nb)
                    for mt4 in range(4):
                        if mt4 == 1 and gi + 1 < len(groups):
                            # next group's panels: fresh pool parity, fires at
                            # emission -> ~25us of lead
                            nhf, nxq, nnb = groups[gi + 1]
                            if (nhf, nxq, nnb) not in wvh:
                                load_wv(nhf, nxq, nnb)
                        if mt4 == 2 and gi % 2 == 0 and gi + 2 < len(groups):
                            # the slab two groups out (other xq parity): its
                            # pool's previous tenant has no readers left after
                            # this group pair, so the DMA fires promptly
                            nhf, nxq, _ = groups[gi + 2]
                            if (nhf, nxq) not in vxh:
                                load_vx(nhf, nxq)
                        mt = xq * 4 + mt4
                        pv = pp.tile([128, 512], FP32, tag="pv", name=f"pv{hf}{nb}{mt}")
                        for k in range(KT):
                            nc.tensor.matmul(
                                pv[:],
                                lhsT=slab[k // 8][:, k % 8, mt4 * 128:(mt4 + 1) * 128],
                                rhs=panels[k // 8][:, k % 8, :],
                                start=(k == 0),
                                stop=(k == KT - 1),
                            )
                        st = hf * 8 + mt
                        # DVE eviction keeps the ACT queue free so the next
                        # slab's DMA triggers fire immediately
                        nc.vector.tensor_copy(
                            v_all[:, st, nb * 512:(nb + 1) * 512], pv[:])

        # ---------- phases 2+3: attention, then output projection ----------
        with ExitStack() as s2:
            c2 = s2.enter_context(tc.tile_pool(name="c2", bufs=1))
            apool = s2.enter_context(tc.tile_pool(name="ap", bufs=1))
            eppool = s2.enter_context(tc.tile_pool(name="ep", bufs=6))
            sqpool = s2.enter_context(tc.tile_pool(name="sq", bufs=4))
            wpool3 = s2.enter_context(tc.tile_pool(name="wp3", bufs=2))
            s2p = s2.enter_context(ExitStack())
            att_ps = s2p.enter_context(tc.tile_pool(name="attps", bufs=2, space="PSUM"))
            av_ps = s2p.enter_context(tc.tile_pool(name="avps", bufs=2, space="PSUM"))
            l_ps = s2p.enter_context(tc.tile_pool(name="lps", bufs=1, space="PSUM"))

            msk_sb = c2.tile([128, 128], BF16, tag="msk", name="msk_sb")
            nc.sync.dma_start(out=msk_sb[:], in_=msk[:])

            attnT = [apool.tile([128, S], BF16, tag=f"attnT{t}", name=f"attnT{t}")
                     for t in range(8)]

            # one l bank pair reused across all j; garbage rows only ever feed
            # unused reciprocal lanes
            lA = l_ps.tile([128, 512], FP32, tag="lA", name="lA")
            lB = l_ps.tile([128, 512], FP32, tag="lB", name="lB")
            nc.vector.memset(lA[:], 1.0)
            nc.vector.memset(lB[:], 1.0)

            # linv tiles allocated up front: their 128-col reciprocal chunks
            # are emitted ONE PER (j,h) GROUP during the next j's groups, so
            # the 0.85us iterative-divide ops never pile up on the DVE queue
            # ahead of the mask/quad ops the PE pipeline depends on.
            linvs = [
                (apool.tile([128, 512], BF16, tag=f"linvA{j}", name=f"linvA{j}"),
                 apool.tile([128, 512], BF16, tag=f"linvB{j}", name=f"linvB{j}"))
                for j in range(4)
            ]
            lsd = {}

            def emit_recip(jsrc, idx):
                bank = 0 if idx < 4 else 1
                cc = (idx % 4) * 128
                src = lsd[(jsrc, bank)]
                dst = linvs[jsrc][bank]
                # The deferred-emission point (next group's score stretch) IS
                # the DVE idle window, so no priority offset: shifting later
                # would land the recip back among that group's masks/quads.
                with nc.allow_low_precision(reason="bf16 1/l: +1e-3 rel err, single-pass bcast matmul"):
                    nc.vector.reciprocal(dst[:, cc:cc + 128], src[:, cc:cc + 128])

            # l-matmuls of group g are emitted inside group g+1 (after its
            # 4th score emission) so the PE never waits on the DVE adds/masks
            # that produce g's quad tiles.  Each entry is a closure.
            pending_lops = []

            for j in range(4):
                ni = 4 * j + 4
                for h in range(8):
                    lbank = lA if h < 4 else lB
                    hp = (h % 4) * 32
                    pav = av_ps.tile([128, 512], FP32, tag="pav", name=f"pav{j}{h}")

                    eps = {}
                    pair_buf = {}
                    diag_lops = []
                    quad_lops = []

                    def c_lo(i, j=j):
                        r = i - 4 * j
                        return 128 * r if r > 0 else 0

                    def emit_score(i, j=j, h=h):
                        # scores land in 2-bank paired PSUM tiles; consecutive
                        # FULL tiles share ONE exp over [128, 2, 512] (ACT op
                        # count 320 -> 224, and the per-op overhead halves on
                        # the bulk).  Diagonal tiles (ragged c0) keep their own
                        # exp on their sub-slice; the strip [c0, c0+128) gets
                        # the triangular mask.
                        c0 = c_lo(i)
                        slot, sub = i // 2, i % 2
                        if sub == 0:
                            psc2 = att_ps.tile([128, 2, 512], FP32, tag="psc",
                                               name=f"psc{j}{h}{slot}")
                            ep2 = eppool.tile([128, 2, 512], BF16, tag="ep",
                                              name=f"ep{j}{h}{slot}")
                            pair_buf[slot] = (psc2, ep2)
                        psc2, ep2 = pair_buf[slot]
                        nc.tensor.matmul(
                            psc2[:, sub, c0:512],
                            lhsT=qkrot[8 + h][:, i * 128:(i + 1) * 128],
                            rhs=qkrot[h][:, j * 512 + c0:(j + 1) * 512],
                            start=True, stop=True,
                        )
                        if c0 > 0:
                            nc.scalar.activation(ep2[:, sub, c0:512],
                                                 psc2[:, sub, c0:512],
                                                 EXP, scale=SCALE_Q)
                        elif sub == 1:
                            # both subs full-width: one exp over the pair
                            nc.scalar.activation(ep2[:, :, :], psc2[:, :, :],
                                                 EXP, scale=SCALE_Q)
                        elif i == 4 * j:
                            # even full-width tile whose partner is ragged
                            nc.scalar.activation(ep2[:, 0, :], psc2[:, 0, :],
                                                 EXP, scale=SCALE_Q)
                        if i - 4 * j >= 0:
                            # triangular mask on the OTHERWISE-IDLE GpSimd:
                            # keeps the in-order DVE queue (recips, quad adds,
                            # evictions) out of the exp->mask->av critical path
                            nc.gpsimd.tensor_mul(ep2[:, sub, c0:c0 + 128],
                                                 ep2[:, sub, c0:c0 + 128],
                                                 msk_sb[:])
                        eps[i] = (ep2, sub)

                    # l reduction plan: full tiles (i < 4j) are pre-summed in
                    # quads on the DVE (one ones-matmul per 4 tiles); the 4
                    # ragged diagonal tiles go straight to the PE.
                    quad = []   # full-width ep tiles awaiting quad reduction
                    nq_flushed = [0]

                    def flush_quad(j=j, h=h):
                        assert len(quad) == 4
                        nq = nq_flushed[0]
                        q0 = sqpool.tile([128, 512], BF16, tag="q0",
                                         name=f"q0_{j}{h}{nq}")
                        q1 = sqpool.tile([128, 512], BF16, tag="q1",
                                         name=f"q1_{j}{h}{nq}")
                        qq = sqpool.tile([128, 512], BF16, tag="qq",
                                         name=f"qq_{j}{h}{nq}")
                        (at_, as_), (bt_, bs_), (ct_, cs_), (dt_, ds_) = quad
                        nc.vector.tensor_add(q0[:], at_[:, as_, :], bt_[:, bs_, :])
                        nc.vector.tensor_add(q1[:], ct_[:, cs_, :], dt_[:, ds_, :])
                        nc.vector.tensor_add(qq[:], q0[:], q1[:])
                        nq_flushed[0] += 1
                        quad.clear()
                        return qq

                    # the group's l-matmul sequence: 4 ragged diag tiles first
                    # (the ii==4j one is full width and carries start=True),
                    # then the quad matmuls, the last carrying stop=True.
                    def add_diag_lop(ep2, sub, c0, ii, j=j, ni=ni, lbank=lbank, hp=hp):
                        def op():
                            nc.tensor.matmul(
                                lbank[hp:hp + 1, c0:512],
                                lhsT=ones_sb[:, 0:1],
                                rhs=ep2[:, sub, c0:512],
                                start=(ii == 4 * j),
                                stop=(j == 0 and ii == ni - 1),
                                tile_position=(0, hp),
                            )
                        diag_lops.append(op)

                    def add_quad_lop(qq, is_last, lbank=lbank, hp=hp):
                        def op():
                            nc.tensor.matmul(
                                lbank[hp:hp + 1, :],
                                lhsT=ones_sb[:, 0:1],
                                rhs=qq[:],
                                start=False, stop=is_last,
                                tile_position=(0, hp),
                            )
                        quad_lops.append(op)

                    # software-pipeline: scores run 4 tiles ahead of l/av so the
                    # exp+mask latency never stalls the PE
                    LOOKAHEAD = 4
                    for i in range(ni + LOOKAHEAD):
                        if i == 4:
                            # PE has ~1us of this group's scores queued: emit
                            # the PREVIOUS group's l-matmuls now
                            for op in pending_lops:
                                op()
                            pending_lops.clear()
                        if i < ni:
                            emit_score(i)
                        ii = i - LOOKAHEAD
                        if ii < 0:
                            continue
                        ep2, sub = eps.pop(ii)
                        c0 = c_lo(ii)
                        if ii < 4 * j:
                            quad.append((ep2, sub))
                            if len(quad) == 4:
                                qq = flush_quad()
                                # the j-th (last) quad of the group ends the
                                # lbank row's accumulation group
                                add_quad_lop(qq, is_last=(ii == 4 * j - 1))
                        else:
                            add_diag_lop(ep2, sub, c0, ii)
                        nc.tensor.matmul(
                            pav[:, c0:512],
                            lhsT=v_all[:, ii, h * 128:(h + 1) * 128],
                            rhs=ep2[:, sub, c0:512],
                            start=(ii == 0), stop=(ii == ni - 1),
                        )
                    assert not quad
                    # execution order: diags first (ii==4j carries start=True),
                    # then quads (last quad carries stop for j>0)
                    pending_lops = diag_lops + quad_lops

                    # DVE copy: keeps the ScalarE exp-only during attention (no
                    # activation-table thrash between Copy and Exp)
                    nc.vector.tensor_copy(attnT[h][:, j * 512:(j + 1) * 512], pav[:])
                    # evict each l bank right after its LAST writer (lA: h==3,
                    # lB: h==7): deferred together with the l-matmuls
                    if h == 3:
                        def evA(j=j):
                            lsA = apool.tile([128, 512], FP32, tag=f"lsA{j}", name=f"lsA{j}")
                            nc.vector.tensor_copy(lsA[:], lA[:])
                            lsd[(j, 0)] = lsA
                        pending_lops.append(evA)
                    elif h == 7:
                        def evB(j=j):
                            lsB = apool.tile([128, 512], FP32, tag=f"lsB{j}", name=f"lsB{j}")
                            nc.vector.tensor_copy(lsB[:], lB[:])
                            lsd[(j, 1)] = lsB
                        pending_lops.append(evB)
                    # one reciprocal chunk per group, spread so they never
                    # head-of-line-block the DVE.  Emitted as deferred closures
                    # since lsd entries appear one group later now.
                    if j > 0:
                        def rec(j=j, h=h):
                            emit_recip(j - 1, h)
                        pending_lops.append(rec)
                    if j == 3 and h >= 4:
                        def rec2(h=h):
                            emit_recip(3, h - 4)
                        pending_lops.append(rec2)

            # flush the final group's deferred ops; the remaining bank-B
            # reciprocals are interleaved into the bc pass below (after j=1)
            # so they never head-of-line-block the bc normalize muls
            for op in pending_lops:
                op()
            pending_lops.clear()

            # prefetch the first o-proj weight panels under the bc pass
            wo_cache = {}

            def get_wop(nb):
                if nb not in wo_cache:
                    lo = wpool3.tile([128, 4, 512], BF16, tag="woplo",
                                     name=f"woplo{nb}")
                    hi = wpool3.tile([128, 4, 512], BF16, tag="wophi",
                                     name=f"wophi{nb}")
                    nc.scalar.dma_start(out=lo[:, :, :], in_=wo[nb, :, 0:4, :])
                    nc.scalar.dma_start(out=hi[:, :, :], in_=wo[nb, :, 4:8, :])
                    wo_cache[nb] = (lo, hi)
                return wo_cache[nb]

            get_wop(0)

            # ---- phase 3: normalize (bc) fused into the output projection --
            # bc(j) broadcasts 1/l across partitions and scales attnT in
            # place; emitting it just-in-time between the first po chains
            # keeps the PE busy while the muls/recips drain on DVE+GpSimd.
            s2p.close()
            with ExitStack() as s3:
                bc_ps = s3.enter_context(tc.tile_pool(name="bcps", bufs=2, space="PSUM"))
                ev3 = s3.enter_context(tc.tile_pool(name="ev3", bufs=4))
                po_ps = s3.enter_context(tc.tile_pool(name="pops", bufs=2, space="PSUM"))

                def emit_bc(j):
                    if j == 1:
                        # bank-B j=3 recips: they only gate bc(3), ~10 po
                        # chains later
                        for idx in range(4, 8):
                            emit_recip(3, idx)
                    for h in range(8):
                        linv = linvs[j][0] if h < 4 else linvs[j][1]
                        hp = (h % 4) * 32
                        bc = bc_ps.tile([128, 512], FP32, tag="bc", name=f"bc{j}{h}")
                        nc.tensor.matmul(
                            bc[:],
                            lhsT=ones_sb[hp:hp + 1, :],
                            rhs=linv[hp:hp + 1, :],
                            start=True, stop=True,
                            tile_position=(hp, 0),
                        )
                        # the in-place scales drain on the DVE while the PE
                        # runs po chains; 8 muls (~5us) per block vs >=7us of
                        # interleaved po work, so the rotation never stalls
                        nc.vector.tensor_mul(
                            attnT[h][:, j * 512:(j + 1) * 512],
                            attnT[h][:, j * 512:(j + 1) * 512],
                            bc[:],
                        )

                emit_bc(0)
                for nb in range(8):
                    # wop rides the Scalar HWDGE queue so it is never stuck
                    # behind the output-tile writes
                    woplo, wophi = get_wop(nb)
                    for mt in range(16):
                        if nb == 0 and mt in (1, 5, 9):
                            emit_bc(mt // 4 + 1)
                        po = po_ps.tile([128, 512], FP32, tag="po", name=f"po{nb}{mt}")
                        for k in range(8):
                            wsrc = woplo if k < 4 else wophi
                            nc.tensor.matmul(
                                po[:],
                                lhsT=attnT[k][:, mt * 128:(mt + 1) * 128],
                                rhs=wsrc[:, k % 4, :],
                                start=(k == 0), stop=(k == 7),
                            )
                        if mt == 0 and nb + 1 < 8:
                            get_wop(nb + 1)
                        osb = ev3.tile([128, 512], FP32, tag="osb", name=f"osb{nb}{mt}")
                        nc.scalar.copy(osb[:], po[:])
                        # 32MB of f32 partials: alternate HWDGE queues so
                        # neither saturates and backpressures the evict pool
                        dq = nc.sync if mt % 2 == 0 else nc.scalar
                        dq.dma_start(
                            out=out[mt * 128:(mt + 1) * 128, nb * 512:(nb + 1) * 512],
                            in_=osb[:],
                        )

    nc.finalize()
    return nc


def _rope_tables(pos_row):
    """cos/sinn tables [128, S]: row p uses inv_freq[p % 64]; the 1/(SX*SW)
    fp8 descale for q,k is folded in.  sinn rows 0-63 are NEGATED so the DVE
    half-swap (t2[0:64] = qkraw[64:128] * sinn[0:64]) carries the rotation
    sign without a separate table."""
    inv = 1.0 / (ROPE_BASE ** (np.arange(0, HD, 2, dtype=np.float32) / HD))  # [64]
    inv128 = np.concatenate([inv, inv]).astype(np.float32)                   # [128]
    ang = inv128[:, None] * pos_row[None, :].astype(np.float32)              # [128, S]
    ds = SQK / (SX * SW)
    sgn = np.concatenate([-np.ones(64, np.float32), np.ones(64, np.float32)])
    return ((np.cos(ang) * ds).astype(bf16),
            (np.sin(ang) * ds * sgn[:, None]).astype(bf16))


def _consts():
    # triangular tile mask: msk[p, c] = 1 iff c >= p
    msk = np.triu(np.ones((128, 128), np.float32))
    ones = np.ones((128, 128), np.float32)
    return msk.astype(bf16), ones.astype(bf16)


def prep_in_maps(hidden_states, w_pack, w_o, positions):
    hidden_states = np.asarray(hidden_states, dtype=np.float32)
    w_pack = np.asarray(w_pack, dtype=np.float32)
    w_o = np.asarray(w_o, dtype=np.float32)
    positions = np.asarray(positions)

    msk, ones = _consts()
    in_maps = []
    for c in range(8):
        b, g = divmod(c, 4)
        # All layouts are slab/panel-major with the partition dim outermost
        # under the panel index, so every DMA reads 2-16KB CONTIGUOUS per
        # partition line (strided 128-512B lines measured ~148GB/s/queue).
        xT = np.ascontiguousarray(hidden_states[b].T)                  # [H, S]
        x_np = np.ascontiguousarray(
            xT.astype(bf16).reshape(KT, 128, 2, 2, 512).transpose(2, 3, 1, 0, 4))
        x8_np = np.ascontiguousarray(
            np.clip(xT * SX, -240, 240).astype(f8)
            .reshape(KT, 128, 2, 2, 512).transpose(2, 3, 1, 0, 4))
        qbase = g * 1024
        kbase = H + g * 1024
        vbase = 2 * H + g * 1024
        wqk_np = np.empty((16, 128, KT, 128), f8)
        for mt in range(16):
            base = qbase + 128 * mt if mt < 8 else kbase + 128 * (mt - 8)
            blk = w_pack[base:base + 128, :]                      # [128, H]
            wqk_np[mt] = (np.clip(blk.T * SW, -240, 240).astype(f8)
                          .reshape(KT, 128, 128).transpose(1, 0, 2))
        wv_np = np.empty((4, 128, 16, 512), bf16)
        for nb in range(2):
            blk = w_pack[vbase + 512 * nb: vbase + 512 * (nb + 1), :]  # [512, H]
            arr = blk.T.astype(bf16).reshape(2, 16, 128, 512)          # [kh, kk, p, c]
            wv_np[2 * nb] = arr[0].transpose(1, 0, 2)
            wv_np[2 * nb + 1] = arr[1].transpose(1, 0, 2)
        woT = np.ascontiguousarray(w_o[:, g * 1024:(g + 1) * 1024].T)  # [1024, H]
        wo_np = np.ascontiguousarray(
            woT.reshape(8, 128, 8, 512).transpose(2, 1, 0, 3)
        ).astype(bf16)
        cos_np, sinn_np = _rope_tables(positions[b])
        in_maps.append({
            "x": x_np, "x8": x8_np, "wqk": wqk_np, "wv": wv_np, "wo": wo_np,
            "cos": cos_np, "sinn": sinn_np,
            "msk": msk, "ones": ones,
        })
    return in_maps


def kernel(hidden_states, w_pack, w_o, positions, _run_kwargs=None):
    if "nc" not in _NC_CACHE:
        _NC_CACHE["nc"] = build_nc()
    nc = _NC_CACHE["nc"]
    in_maps = prep_in_maps(hidden_states, w_pack, w_o, positions)
    res = run_bass_kernel_spmd(nc, in_maps, core_ids=list(range(8)),
                               **(_run_kwargs or {}))
    _NC_CACHE["last_result"] = res
    out = np.zeros((B, S, H), np.float32)
    for c in range(8):
        b = c // 4
        out[b] += res.results[c]["out"]
    return out
